# revision 45
# baseline (speedup 1.0000x reference)
"""Trainium2 Bass kernel for nn_Block_6579889898195 (ragged_sequence).

Self-contained: hardcodes shapes/sharding. Data-parallel over batch across 8
NeuronCores; the two global reductions (mean/std of x, batchnorm batch stats)
are AllReduce collectives.

Mathematical restructuring (validated against the reference in numpy):
  * the joint_explict interleave + dilated conv collapses into three dense
    stride-2 convs P/Q/Q' over the odd/even phases of x, with the 0.8/mean and
    0.8/std scalings folded into an affine recombination of P and Q
  * maxpool is computed from the even/odd split of the conv output
  * the dpadding gather is folded into weighted batchnorm sums on the compact
    grid plus a final composed gather
  * the output permutation (joint/sew_up/dpad composition) is precomputed on
    the host into ~32 strided access-pattern families
"""
import os
import sys
import threading

import numpy as np

for _p in ("/opt/trn_rl_repo", "/root/.axon_site/_ro/trn_rl_repo"):
    if os.path.isdir(_p) and _p not in sys.path:
        sys.path.insert(0, _p)

import concourse.bass as bass
import concourse.tile as tile
from concourse import bacc, mybir
from concourse.bass_utils import run_bass_kernel_spmd
from concourse.masks import make_identity

dt = mybir.dt
Alu = mybir.AluOpType
Act = mybir.ActivationFunctionType
Ax = mybir.AxisListType

N_CORES = 8
B, T, C = 64, 4096, 64
BPC = B // N_CORES          # batches per core
PAIRS = BPC // 2
KERNEL, STRIDE, DILATION, BN_EPS = 3, 2, 3, 1e-5
TH = T // 2                 # 2048, width of XN/PV tiles

# block geometry: (M, YL, PL, EW, dup_hi, Ndp, alloc_w, xn_off, xn_str, pv_off, pv_str, qp_off)
# A: conv over xn=xT[1::2], pv=xT[0::2]; rhs XN[2m+k], PV[2m+k], PV[2m+1+k]
# B: xn=xT[3::4]=XN[2i+1], pv=xT[0::4]=PV[2i]; rhs XN[4m+2k+1], PV[4m+2k], PV[4m+2k+2]
BLK_A = dict(M=1023, YL=3069, PL=1535, EW=1536, dup_hi=1026, Ndp=B * 2048, AW=1544,
             xn0=lambda k: k, xns=2, pv0=lambda k: k, pvs=2, qp0=lambda k: k + 1)
BLK_B = dict(M=511, YL=1533, PL=767, EW=768, dup_hi=514, Ndp=B * 1024, AW=776,
             xn0=lambda k: 2 * k + 1, xns=4, pv0=lambda k: 2 * k, pvs=4, qp0=lambda k: 2 * k + 2)

NTOT = B * T * C  # global element count of x


# ---------------------------------------------------------------------------
# static index plan (host side)
# ---------------------------------------------------------------------------

def _sew_up_indices(a_len, b_len, cur_layer, sp, st):
    idx = []
    cnt_a = cnt_b = 0
    while cnt_a < a_len:
        if sp:
            break
        if cnt_a == 0:
            pv = cnt_b
            for _ in range(st['skip_p']):
                idx.append(a_len + pv)
                cnt_b += 1
            if cur_layer % 2 != 0:
                idx.append(0)
                cnt_a += 1
            if cnt_a == 0:
                cnt_a = 1
            continue
        for _ in range(st['skip_n']):
            if cnt_b >= b_len:
                break
            idx.append(a_len + cnt_b)
            cnt_b += 1
        if not st['skip_t']:
            for _ in range(st['skip_s']):
                if cnt_b >= b_len:
                    break
                idx.append(a_len + cnt_b)
                cnt_b += 1
                st['skip_t'] = True
        else:
            for _j in range(st['skip_d']):
                for _i in range(st['skip_s']):
                    if cnt_b >= b_len:
                        break
                    idx.append(a_len + cnt_b)
                    cnt_b += 1
                if cnt_a >= a_len:
                    break
                idx.append(cnt_a)
                cnt_a += 1
                st['skip_t'] = False
            continue
        idx.append(cnt_a)
        cnt_a += 1
    idx += [a_len + j for j in range(cnt_b, b_len)]
    if sp:
        idx += list(range(cnt_a, a_len))
    st['skip_s'] += 1
    st['skip_n'] = 3 * st['skip_n'] + st['skip_s'] + st['skip_d'] * st['skip_s']
    if cur_layer % 2 != 0:
        st['skip_p'] += 1
    return np.asarray(idx, np.int64)


def _dpadding_indices(a_len, num_padding):
    if num_padding == 0:
        return np.arange(a_len, dtype=np.int64)
    skip_cnt = a_len // num_padding
    mult = None
    if skip_cnt == 0:
        mult = num_padding // a_len
        skip_cnt = 1
    entries = []
    rem = num_padding
    for i in range(a_len):
        if rem == 0:
            entries.append(list(range(i, a_len)))
            break
        if i % skip_cnt == 0:
            entries.append([i])
            rem -= 1
        if mult is not None:
            entries.extend([[i]] * mult)
        entries.append([i])
    return np.asarray([k for e in entries for k in e], np.int64)


def _build_final_map():
    st = dict(skip_p=1, skip_s=1, skip_d=2, skip_n=3, skip_t=False)
    si = _sew_up_indices(2048, 4096, 1, False, st)
    pi_A = _dpadding_indices(1535, 513)
    pi_B = _dpadding_indices(767, 257)
    U = len(si)
    kind = np.empty(U, np.int64)
    xi = np.empty(U, np.int64)
    ui = np.full(U, -1, np.int64)
    for u, s in enumerate(si):
        if s < 2048:
            t = int(s)
            if t % 2 == 0:
                pj = (4 * (t // 2)) // 2         # PV column
                kind[u] = 0 if pj % 2 == 0 else 3
                xi[u] = pj // 2                  # PVE/PVO column
            else:
                j = (t - 1) // 2
                kind[u] = 2
                xi[u] = j                        # XNo column (xn col 2j+1)
                ui[u] = pi_B[j]
        else:
            r = int(s) - 2048
            if r % 2 == 0:
                pj = r // 2
                kind[u] = 0 if pj % 2 == 0 else 3
                xi[u] = pj // 2
            else:
                j = (r - 1) // 2
                kind[u] = 1 if j % 2 == 0 else 4  # XNe / XNo column
                xi[u] = j // 2
                ui[u] = pi_A[j]
    return kind, xi, ui


def _plan_ops():
    kind, xi, ui = _build_final_map()
    U = len(kind)
    used = np.zeros(U, bool)
    ops = []
    for k in (0, 3, 1, 4, 2):
        usl = np.where(kind == k)[0].tolist()
        pos = set(usl)
        idx_of = {u: i for i, u in enumerate(usl)}
        for u0 in usl:
            if used[u0]:
                continue
            best = None
            cands = []
            i0 = idx_of[u0]
            for nxt in usl[i0 + 1:i0 + 40]:
                if not used[nxt]:
                    cands.append(nxt - u0)
                if len(cands) >= 30:
                    break
            for du in cands:
                dx0 = xi[u0 + du] - xi[u0] if (u0 + du) in pos else None
                if dx0 is None or dx0 <= 0:
                    continue
                if k in (1, 4, 2) and ui[u0 + du] - ui[u0] <= 0:
                    continue
                cnt = 1
                u = u0
                while True:
                    un = u + du
                    if un >= U or un not in pos or used[un]:
                        break
                    if xi[un] - xi[u] != dx0:
                        break
                    if k in (1, 4, 2) and (ui[un] - ui[u] != ui[u0 + du] - ui[u0]):
                        break
                    u = un
                    cnt += 1
                if best is None or cnt > best[1]:
                    best = (du, cnt)
            du, cnt = best if best else (1, 1)
            if cnt == 1:
                ops.append((k, u0, 1, 1, int(xi[u0]), 0, int(ui[u0]), 0))
                used[u0] = True
                continue
            dx = int(xi[u0 + du] - xi[u0])
            dui = int(ui[u0 + du] - ui[u0]) if k in (1, 4, 2) else 0
            for t in range(cnt):
                used[u0 + t * du] = True
            ops.append((k, int(u0), int(du), int(cnt), int(xi[u0]), dx, int(ui[u0]), dui))
    assert used.all()
    return ops


def _sl(tileap, start, step, count):
    """Strided free-dim slice [start : start+(count-1)*step+1 : step]."""
    if count == 1 or step == 1:
        return tileap[:, start:start + count]
    assert step > 0
    return tileap[:, start:start + (count - 1) * step + 1:step]


# ---------------------------------------------------------------------------
# program builder
# ---------------------------------------------------------------------------

def _build_program(debug=False):
    KLEVEL = int(os.environ.get("KLEVEL", "3"))
    KPARTS = int(os.environ.get("KPARTS", "7"))
    nc = bacc.Bacc(num_devices=N_CORES)
    f32, f32r, f16 = dt.float32, dt.float32r, dt.float16

    x_t = nc.dram_tensor("x", [BPC, T, C], f32, kind="ExternalInput")
    v_t = nc.dram_tensor("conv_v", [C, C, KERNEL], f32, kind="ExternalInput")
    g_t = nc.dram_tensor("conv_g", [C], f32, kind="ExternalInput")
    b_t = nc.dram_tensor("conv_b", [C], f32, kind="ExternalInput")
    gam_t = nc.dram_tensor("bn_gamma", [C], f32, kind="ExternalInput")
    bet_t = nc.dram_tensor("bn_beta", [C], f32, kind="ExternalInput")
    out_t = nc.dram_tensor("out", [BPC, C, 6144], f32, kind="ExternalOutput")
    dbg = {}
    if debug:
        dbg['g_A'] = nc.dram_tensor("dbg_g_A", [128, 1544], dt.float16, kind="ExternalOutput")
        dbg['u_A'] = nc.dram_tensor("dbg_u_A", [128, 1544], dt.float16, kind="ExternalOutput")
        dbg['stats8'] = nc.dram_tensor("dbg_stats8", [64, 8], f32, kind="ExternalOutput")
        dbg['ac'] = nc.dram_tensor("dbg_ac", [1, 2], f32, kind="ExternalOutput")
        dbg['yA'] = nc.dram_tensor("dbg_yA", [128, 1544], dt.float16, kind="ExternalOutput")
        dbg['yoA'] = nc.dram_tensor("dbg_yoA", [128, 1544], dt.float16, kind="ExternalOutput")
        dbg['poolA'] = nc.dram_tensor("dbg_poolA", [128, 1544], dt.float16, kind="ExternalOutput")
        dbg['zA'] = nc.dram_tensor("dbg_zA", [128, 1544], dt.float16, kind="ExternalOutput")

    plan = _plan_ops()
    CH = 3072

    with tile.TileContext(nc) as tc:
        import contextlib
        ctx = contextlib.ExitStack()
        with ctx:
            const = ctx.enter_context(tc.tile_pool(name="const", bufs=1))
            xload_p = ctx.enter_context(tc.tile_pool(name="xload", bufs=2))
            xn_p = ctx.enter_context(tc.tile_pool(name="xn", bufs=PAIRS))
            pv_p = ctx.enter_context(tc.tile_pool(name="pv", bufs=PAIRS))
            y_p = ctx.enter_context(tc.tile_pool(name="y", bufs=1))
            pool_p = ctx.enter_context(tc.tile_pool(name="pool", bufs=1))
            tmp_p = ctx.enter_context(tc.tile_pool(name="tmp", bufs=2))
            gu_p = ctx.enter_context(tc.tile_pool(name="gu", bufs=PAIRS))
            pv0_p = ctx.enter_context(tc.tile_pool(name="pv0", bufs=1))
            qsb_p = ctx.enter_context(tc.tile_pool(name="qsb", bufs=4))
            stat_p = ctx.enter_context(tc.tile_pool(name="stat", bufs=2 * PAIRS))
            stage_p = ctx.enter_context(tc.tile_pool(name="stage", bufs=3))
            tp_ps = ctx.enter_context(tc.tile_pool(name="tp_ps", bufs=2, space="PSUM"))
            conv_ps = ctx.enter_context(tc.tile_pool(name="conv_ps", bufs=5, space="PSUM"))
            misc_ps = ctx.enter_context(tc.tile_pool(name="misc_ps", bufs=1, space="PSUM"))
            dram = ctx.enter_context(tc.tile_pool(name="dram", bufs=8, space="DRAM"))

            # ---------------- constants & parameters ----------------
            ident = const.tile([128, 128], f32, tag="ident")
            make_identity(nc, ident[:])
            ones = const.tile([128, 1], f32, tag="ones")
            nc.vector.memset(ones[:], 1.0)
            ii = const.tile([128, 64], f32, tag="ii")
            make_identity(nc, ii[0:64, :])
            make_identity(nc, ii[64:128, :])

            bias128 = const.tile([128, 1], f32, tag="bias128")
            nc.sync.dma_start(out=bias128[0:64, :], in_=b_t[:])
            nc.sync.dma_start(out=bias128[64:128, :], in_=b_t[:])
            gamma_sb = const.tile([64, 1], f32, tag="gamma")
            nc.sync.dma_start(out=gamma_sb[:], in_=gam_t[:])
            beta_sb = const.tile([64, 1], f32, tag="beta")
            nc.sync.dma_start(out=beta_sb[:], in_=bet_t[:])
            eps_sb = const.tile([64, 1], f32, tag="eps")
            nc.vector.memset(eps_sb[:], BN_EPS)

            # ---------------- weight prep ----------------
            do_wp = KPARTS & 4
            vt = const.tile([64, 192], f32, tag="vt")
            nc.sync.dma_start(out=vt[:], in_=v_t[:])
            gg = const.tile([64, 1], f32, tag="gg")
            nc.sync.dma_start(out=gg[:], in_=g_t[:])
            dumw = const.tile([64, 192], f32, tag="dumw")
            nrm2 = const.tile([64, 4], f32, tag="nrm2")
            lhsP = const.tile([128, 3, 128], f32, tag="lhsP")
            lhsPr = const.tile([128, 3, 128], f16, tag="lhsPr")
            lhsQ = const.tile([128, 3, 128], f16, tag="lhsQ")
            wsc = const.tile([64, 192], f32, tag="wsc")
            if do_wp:
                nc.vector.tensor_mul(out=dumw[:], in0=vt[:], in1=vt[:])
                nc.vector.tensor_reduce(out=nrm2[:, 0:1], in_=dumw[:], axis=Ax.X, op=Alu.add)
                nc.scalar.activation(out=nrm2[:, 1:2], in_=nrm2[:, 0:1], func=Act.Sqrt)
                nc.vector.reciprocal(out=nrm2[:, 2:3], in_=nrm2[:, 1:2])
                nc.vector.tensor_mul(out=nrm2[:, 3:4], in0=gg[:], in1=nrm2[:, 2:3])
                nc.vector.tensor_scalar_mul(out=wsc[:], in0=vt[:], scalar1=nrm2[:, 3:4])
                KWP = int(os.environ.get("KWP", "3"))
                nc.vector.memset(lhsP[:], 0.0)
                if KWP >= 2:
                    pw = misc_ps.tile([128, 512], f32, tag="misc")
                    for k in range(3):
                        nc.tensor.transpose(pw[0:64, 64 * k:64 * k + 64], wsc[:, k:192:3], ident[0:64, 0:64])
                    if KWP >= 3:
                        for k in range(3):
                            nc.vector.tensor_copy(out=lhsP[0:64, k, 0:64], in_=pw[0:64, 64 * k:64 * k + 64])
                            nc.vector.tensor_copy(out=lhsP[64:128, k, 64:128], in_=pw[0:64, 64 * k:64 * k + 64])
                nc.vector.tensor_copy(out=lhsPr[:], in_=lhsP[:])
                nc.scalar.mul(out=lhsQ[:], in_=lhsP[:], mul=0.2)
            else:
                nc.vector.memset(lhsP[:], 0.0)
                nc.vector.tensor_copy(out=lhsPr[:], in_=lhsP[:])
                nc.vector.tensor_copy(out=lhsQ[:], in_=lhsP[:])

            # ---------------- loads + x-stats ----------------
            # Per pair of batches, DMA three strided t-streams directly from
            # HBM (xn = odd t, pve = t%4==0, pvo = t%4==2), packing the two
            # batches' channels side by side in the free dim so one [128,128]
            # PE transpose handles both batches at once.
            xbst = const.tile([128, 4 * BPC, 6], f32, tag="xbst")
            lload = []
            for p in range(PAIRS):
                lxne = xload_p.tile([128, 8, 128], f32, tag="lxne")
                lxno = xload_p.tile([128, 8, 128], f32, tag="lxno")
                lpve = xload_p.tile([128, 8, 128], f32, tag="lpve")
                lpvo = xload_p.tile([128, 8, 128], f32, tag="lpvo")
                # one DMA per stream per batch; issue queues spread across
                # engines so descriptor generation doesn't serialize on Sync
                for dlt in range(2):
                    b = 2 * p + dlt
                    cs = slice(64 * dlt, 64 * dlt + 64)
                    nc.sync.dma_start(
                        out=lxne[:, :, cs],
                        in_=x_t[b].rearrange("(n p q) c -> q p n c", p=128, q=4)[1])
                    nc.sync.dma_start(
                        out=lxno[:, :, cs],
                        in_=x_t[b].rearrange("(n p q) c -> q p n c", p=128, q=4)[3])
                    nc.scalar.dma_start(
                        out=lpve[:, :, cs],
                        in_=x_t[b].rearrange("(n p q) c -> q p n c", p=128, q=4)[0])
                    nc.gpsimd.dma_start(
                        out=lpvo[:, :, cs],
                        in_=x_t[b].rearrange("(n p q) c -> q p n c", p=128, q=4)[2])
                if KPARTS & 1:
                    chk = 0
                    for src, nch in ((lxne, 2), (lxno, 2), (lpve, 2), (lpvo, 2)):
                        f = src[:].rearrange("p n c -> p (n c)")
                        for c0 in range(nch):
                            nc.vector.bn_stats(out=xbst[:, 8 * p + chk, :],
                                               in_=f[:, 512 * c0:512 * (c0 + 1)])
                            chk += 1
                lload.append((lxne, lxno, lpve, lpvo))
            if not (KPARTS & 1):
                nc.vector.memset(xbst[:], 0.5)
            xmv = const.tile([128, 8], f32, tag="xmv")
            nc.vector.bn_aggr(out=xmv[:, 0:2], in_=xbst[:])
            # sums per partition: s = mean*16384 ; ss = (var + mean^2)*16384
            nc.vector.tensor_mul(out=xmv[:, 2:3], in0=xmv[:, 0:1], in1=xmv[:, 0:1])
            nc.vector.tensor_add(out=xmv[:, 3:4], in0=xmv[:, 1:2], in1=xmv[:, 2:3])
            xsums = const.tile([128, 2], f32, tag="xsums")
            nc.vector.tensor_scalar_mul(out=xsums[:, 0:1], in0=xmv[:, 0:1], scalar1=float(T * C * BPC // 128))
            nc.vector.tensor_scalar_mul(out=xsums[:, 1:2], in0=xmv[:, 3:4], scalar1=float(T * C * BPC // 128))
            pf1 = misc_ps.tile([128, 512], f32, tag="misc")
            nc.tensor.matmul(pf1[0:1, 0:2], ones[:], xsums[:], start=True, stop=True)
            s12 = const.tile([1, 2], f32, tag="s12")
            nc.vector.tensor_copy(out=s12[:], in_=pf1[0:1, 0:2])
            ar1_in = dram.tile([1, 2], f32, tag="ar1i")
            ar1_out = dram.tile([1, 2], f32, tag="ar1o")
            nc.sync.dma_start(out=ar1_in[:], in_=s12[:])
            nc.gpsimd.collective_compute(
                "AllReduce", Alu.add, replica_groups=[list(range(N_CORES))],
                ins=[ar1_in[:].opt()], outs=[ar1_out[:].opt()])


            # ---------------- transposes -> XN / PV per pair ----------------
            if KLEVEL < 1:
                raise SystemExit
            XNE, XNO, XNE1, PVE, PVO, PV00, PV02 = [], [], [], [], [], [], []
            f32r = dt.float32r
            for p in range(PAIRS):
                xne = xn_p.tile([128, TH // 2], f32, tag="xne")
                xno = xn_p.tile([128, TH // 2], f32, tag="xno")
                pve = pv_p.tile([128, TH // 2], f16, tag="pve")
                pvo = pv_p.tile([128, TH // 2], f16, tag="pvo")
                lxne, lxno, lpve, lpvo = lload[p]
                di = p  # stagger drain-engine rotation across pairs
                for (lt, nblk, dst) in ((lxne, 8, xne), (lxno, 8, xno),
                                        (lpve, 8, pve), (lpvo, 8, pvo)):
                    if not (KPARTS & 2):
                        break
                    for s in range(nblk // 4):
                        ptp = tp_ps.tile([128, 512], f32, tag="tp")
                        for q in range(4):
                            nc.tensor.transpose(ptp[:, 128 * q:128 * (q + 1)],
                                                lt[:, 4 * s + q, :], ident[:])
                        dsl = dst[:, 512 * s:512 * (s + 1)]
                        # gpsimd cannot read PSUM; alternate scalar/vector
                        if di % 2 == 0:
                            nc.scalar.copy(out=dsl, in_=ptp[:])
                        else:
                            nc.vector.tensor_copy(out=dsl, in_=ptp[:])
                        di += 1
                pv00 = pv0_p.tile([128, TH // 4], f16, tag="pv00")
                pv02 = pv0_p.tile([128, TH // 4], f16, tag="pv02")
                # shifted copy of XNe so the k=2 P-tap is even-aligned for f32r
                xne1 = pv0_p.tile([128, TH // 2], f32, tag="xne1")
                if KPARTS & 2:
                    nc.scalar.copy(out=pv00[:], in_=pve[:, 0:1024:2])
                    nc.vector.tensor_copy(out=pv02[:], in_=pve[:, 1:1024:2])
                    nc.vector.tensor_copy(out=xne1[:, 0:1023].bitcast(f32),
                                          in_=xne[:, 1:1024].bitcast(f32))
                    nc.vector.memset(xne1[:, 1023:1024].bitcast(f32), 0.0)
                XNE.append(xne)
                XNO.append(xno)
                XNE1.append(xne1)
                PVE.append(pve)
                PVO.append(pvo)
                PV00.append(pv00)
                PV02.append(pv02)

            # ---------------- assembly helpers + pair-0 passthrough ----------
            # kind 0/3 copies depend only on PVE/PVO; emitting pair 0's early
            # fills engine idle time during the AR1 collective.
            eng_acc = {'v': 8000.0, 's': 14000.0, 'g': 0.0}

            def _op_cost(e, n):
                return {'v': n * 1.04 + 105.0, 's': n * 0.83 + 124.0,
                        'g': n * 1.05 + 131.0}[e]

            def _pick(cand, n):
                e = min(cand, key=lambda e: eng_acc[e] + _op_cost(e, n))
                eng_acc[e] += _op_cost(e, n)
                return e

            def _emit_ops(p, ci, stage, which):
                cs_u, ce_u = ci * CH, (ci + 1) * CH
                for (k, u0, du, cnt, x0, dx, ui0, dui) in plan:
                    if (k in (0, 3)) != (which == 'copy'):
                        continue
                    t0 = max(0, -(-(cs_u - u0) // du))
                    t1 = min(cnt - 1, (ce_u - 1 - u0) // du)
                    if t1 < t0:
                        continue
                    n = t1 - t0 + 1
                    us = u0 + t0 * du - cs_u
                    xs = x0 + t0 * dx
                    if k in (0, 3):
                        src = PVE[p][:] if k == 0 else PVO[p][:]
                        e = _pick(('s', 'g', 'v'), n)
                        fn = {'s': nc.scalar.copy, 'g': nc.gpsimd.tensor_copy,
                              'v': nc.vector.tensor_copy}[e]
                        fn(out=_sl(stage, us, du, n), in_=_sl(src, xs, dx, n))
                    else:
                        uu = ui0 + t0 * dui
                        ut = u_tiles[(p, 0 if k in (1, 4) else 1)]
                        srcx = (XNE[p] if k == 1 else XNO[p])[:].bitcast(f32)
                        e = _pick(('v', 'g'), n)
                        eng = nc.vector if e == 'v' else nc.gpsimd
                        eng.tensor_add(out=_sl(stage, us, du, n),
                                       in0=_sl(srcx, xs, dx, n),
                                       in1=_sl(ut, uu, dui, n))

            stage_tiles = {}
            if KLEVEL >= 3:
                for (pp, ci) in ((0, 0), (0, 1), (1, 0)):
                    stage = stage_p.tile([128, CH], f32, tag="stage")
                    stage_tiles[(pp, ci)] = stage
                    _emit_ops(pp, ci, stage, 'copy')

            # a/c chain emitted late so its AR1 wait does not block queues
            gs = const.tile([1, 12], f32, tag="gs")
            nc.sync.dma_start(out=gs[:, 0:2], in_=ar1_out[:])
            # a = 0.8*N/S1 ; c = 0.8/std, std = sqrt((S2 - S1^2/N)/(N-1))
            nc.vector.reciprocal(out=gs[:, 2:3], in_=gs[:, 0:1])
            nc.vector.tensor_scalar_mul(out=gs[:, 3:4], in0=gs[:, 2:3], scalar1=float(0.8 * NTOT))  # a
            nc.vector.tensor_scalar_mul(out=gs[:, 4:5], in0=gs[:, 0:1], scalar1=float(1.0 / NTOT))
            nc.vector.tensor_mul(out=gs[:, 5:6], in0=gs[:, 0:1], in1=gs[:, 4:5])
            nc.vector.tensor_sub(out=gs[:, 6:7], in0=gs[:, 1:2], in1=gs[:, 5:6])
            nc.vector.tensor_scalar_mul(out=gs[:, 7:8], in0=gs[:, 6:7], scalar1=float(1.0 / (NTOT - 1)))
            nc.scalar.activation(out=gs[:, 8:9], in_=gs[:, 7:8], func=Act.Sqrt)
            nc.vector.reciprocal(out=gs[:, 9:10], in_=gs[:, 8:9])
            nc.vector.tensor_scalar_mul(out=gs[:, 10:11], in0=gs[:, 9:10], scalar1=0.8)  # c
            ac_pack = const.tile([1, 2], f32, tag="acp")
            nc.vector.tensor_copy(out=ac_pack[:, 0:1], in_=gs[:, 3:4])
            nc.vector.tensor_copy(out=ac_pack[:, 1:2], in_=gs[:, 10:11])
            a128 = const.tile([128, 1], f32, tag="a128")
            c128 = const.tile([128, 1], f32, tag="c128")
            if int(os.environ.get("KNOBCAST", "0")):
                nc.vector.memset(a128[:], 1.0)
                nc.vector.memset(c128[:], 1.0)
            else:
                nc.gpsimd.partition_broadcast(a128[:], ac_pack[0:1, 0:1])
                nc.gpsimd.partition_broadcast(c128[:], ac_pack[0:1, 1:2])
            if debug:
                nc.sync.dma_start(out=dbg['ac'][:], in_=ac_pack[:])

            # ---------------- conv + pointwise per pair/block ----------------
            do_conv = KLEVEL >= 2
            g_tiles = {}   # (p, blk) -> g tile (f16)
            st_tiles = {}  # (p, blk) -> [128,4] stats
            for p in range(PAIRS if do_conv else 0):
                qtiles = {0: (PVE[p], PVO[p]), 1: (PV00[p], PV02[p])}
                for blk_i, BL in enumerate((BLK_A, BLK_B)):
                    M, PL, EW, AW = BL['M'], BL['PL'], BL['EW'], BL['AW']
                    tagb = f"b{blk_i}"
                    ye = y_p.tile([128, BLK_A["AW"]], f16, tag="ye")
                    yo = y_p.tile([128, BLK_A["AW"]], f16, tag="yo")
                    nc.gpsimd.memset(yo[:, 0:1], float("-inf"))
                    nc.gpsimd.memset(yo[:, PL:PL + 2], float("-inf"))
                    nc.gpsimd.memset(ye[:, PL:PL + 1], 0.0)
                    halves = [(0, 512, 512), (512, 512, 511)] if blk_i == 0 else [(0, 512, 511)]
                    for (m0, mw, realw) in halves:
                        psP = conv_ps.tile([128, 512], f32, tag="conv")
                        psQ = conv_ps.tile([128, 512], f32, tag="conv")
                        psQp = conv_ps.tile([128, 512], f32, tag="conv")
                        if blk_i == 0:
                            # P_A must be full fp32 (a ~ 5e4 amplifies its
                            # error); f32r turned out to be reduced-precision.
                            # Dense even taps on XNe/XNo/XNe1.
                            taps = (XNE[p], XNO[p], XNE1[p])
                            for k in range(3):
                                nc.tensor.matmul(psP[:, 0:512], lhsP[:, k, :],
                                                 taps[k][:, m0:m0 + 512].bitcast(f32),
                                                 start=(k == 0), stop=(k == 2))
                        else:
                            # P_B: fp32 with stride-2 taps on XNo
                            xnof = XNO[p][:].bitcast(f32)
                            for k in range(3):
                                nc.tensor.matmul(psP[:, 0:realw], lhsP[:, k, :],
                                                 _sl(xnof, k, 2, realw),
                                                 start=(k == 0), stop=(k == 2))
                        qe, qo = qtiles[blk_i]
                        # fp16 taps allow arbitrary column alignment:
                        # psQ  = R_even = 0.2*(W0 qe[m] + W1 qo[m] + W2 qe[m+1])
                        # psQp = R_odd  =      W0 qo[m] + W1 qe[m+1] + W2 qo[m+1]
                        for ps, lt, taps in ((psQ, lhsQ, ((qe, 0), (qo, 0), (qe, 1))),
                                             (psQp, lhsPr, ((qo, 0), (qe, 1), (qo, 1)))):
                            for k, (qt, off) in enumerate(taps):
                                nc.tensor.matmul(ps[:, 0:realw], lt[:, k, :],
                                                 qt[:, m0 + off:m0 + off + realw],
                                                 start=(k == 0), stop=(k == 2))
                        # Drain conv psums to SBUF immediately (AR1-independent)
                        # so PE never stalls on the collective: the a/c-scaled
                        # STT reads these SBUF copies later.
                        qsb = qsb_p.tile([128, 512], f16, tag="qsb")
                        nc.scalar.copy(out=qsb[:, 0:realw], in_=psQ[:, 0:realw])
                        psb = qsb_p.tile([128, 512], f32, tag="psb")
                        nc.scalar.copy(out=psb[:, 0:realw], in_=psP[:, 0:realw])
                        # scatter phases into ye/yo (q = parity of m); ph2 first
                        # (no a/c dependency) so psQp frees early too
                        for ph in (2, 0, 1):
                            for q in range(2):
                                ms = m0 if m0 % 2 == q else m0 + 1
                                if ms >= m0 + realw:
                                    continue
                                cnt = (m0 + realw - ms + 1) // 2
                                l0 = 3 * ms + ph
                                if l0 % 2 == 0:
                                    ytile, ycol = ye, l0 // 2
                                else:
                                    ytile, ycol = yo, (l0 - 1) // 2 + 1
                                pscol = ms - m0
                                if ph == 2:
                                    nc.scalar.copy(out=_sl(ytile, ycol, 3, cnt),
                                                   in_=_sl(psQp, pscol, 2, cnt))
                                else:
                                    sc = a128 if ph == 0 else c128
                                    nc.vector.scalar_tensor_tensor(
                                        out=_sl(ytile, ycol, 3, cnt),
                                        in0=_sl(psb, pscol, 2, cnt), scalar=sc[:],
                                        in1=_sl(qsb, pscol, 2, cnt),
                                        op0=Alu.mult, op1=Alu.add)
                    if debug and p == 0 and blk_i == 0:
                        nc.sync.dma_start(out=dbg['yA'][:], in_=ye[:])
                        nc.sync.dma_start(out=dbg['yoA'][:], in_=yo[:])
                    # ---- pool ----
                    pool = pool_p.tile([128, BLK_A["AW"]], f16, tag="pool")
                    nc.vector.tensor_max(out=pool[:, 0:EW], in0=yo[:, 0:EW], in1=ye[:, 0:EW])
                    nc.vector.tensor_max(out=pool[:, 0:EW], in0=pool[:, 0:EW], in1=yo[:, 1:EW + 1])
                    # ---- v = pool + bias ; chain to g ----
                    vmin = tmp_p.tile([128, BLK_A["AW"]], f16, tag="t1")
                    vmax = tmp_p.tile([128, BLK_A["AW"]], f16, tag="t2")
                    ee = tmp_p.tile([128, BLK_A["AW"]], f16, tag="t3")
                    nc.vector.tensor_scalar(out=vmin[:, 0:EW], in0=pool[:, 0:EW],
                                            scalar1=bias128[:], scalar2=0.0,
                                            op0=Alu.add, op1=Alu.min)
                    nc.vector.tensor_scalar(out=vmax[:, 0:EW], in0=pool[:, 0:EW],
                                            scalar1=bias128[:], scalar2=0.0,
                                            op0=Alu.add, op1=Alu.max)
                    nc.scalar.activation(out=ee[:, 0:EW], in_=vmin[:, 0:EW], func=Act.Exp)
                    # z = vmax - 1 + e  (reuse vmax tile)
                    nc.vector.scalar_tensor_tensor(out=vmax[:, 0:EW], in0=vmax[:, 0:EW],
                                                   scalar=-1.0, in1=ee[:, 0:EW],
                                                   op0=Alu.add, op1=Alu.add)
                    # zsq (reuse vmin)
                    if debug and p == 0 and blk_i == 0:
                        nc.sync.dma_start(out=dbg['poolA'][:], in_=pool[:])
                        nc.sync.dma_start(out=dbg['zA'][:], in_=vmax[:])
                    nc.vector.tensor_mul(out=vmin[:, 0:EW], in0=vmax[:, 0:EW], in1=vmax[:, 0:EW])
                    gt = gu_p.tile([128, AW], f16, tag="g" + tagb)
                    st4 = stat_p.tile([128, 4], f32, tag="st4")
                    nc.scalar.activation(out=gt[:, 0:PL], in_=vmin[:, 0:PL], func=Act.Exp,
                                         scale=-0.5, accum_out=st4[:, 0:1])
                    nc.scalar.activation(out=ee[:, 0:PL], in_=gt[:, 0:PL], func=Act.Square,
                                         accum_out=st4[:, 1:2])
                    dcnt = BL['dup_hi'] // 2
                    nc.scalar.activation(out=vmax[:, 0:dcnt], in_=_sl(gt, 0, 2, dcnt),
                                         func=Act.Identity, accum_out=st4[:, 2:3])
                    nc.scalar.activation(out=vmin[:, 0:dcnt], in_=_sl(gt, 0, 2, dcnt),
                                         func=Act.Square, accum_out=st4[:, 3:4])
                    g_tiles[(p, blk_i)] = gt
                    st_tiles[(p, blk_i)] = st4

            # ---------------- AR2: batchnorm stats ----------------
            do_post = KLEVEL >= 3 and do_conv
            stats8 = const.tile([128, 8], f32, tag="stats8")
            for blk_i in range(2 if do_post else 0):
                cs = 4 * blk_i
                nc.vector.tensor_copy(out=stats8[:, cs:cs + 4], in_=st_tiles[(0, blk_i)][:])
                for p in range(1, PAIRS):
                    nc.vector.tensor_add(out=stats8[:, cs:cs + 4], in0=stats8[:, cs:cs + 4],
                                         in1=st_tiles[(p, blk_i)][:])
            pf2 = misc_ps.tile([128, 512], f32, tag="misc")
            if do_post:
                nc.tensor.matmul(pf2[0:64, 0:8], ii[:], stats8[:], start=True, stop=True)
            st8f = const.tile([64, 8], f32, tag="st8f")
            ar2_sb = const.tile([64, 8], f32, tag="ar2sb")
            if do_post:
                nc.vector.tensor_copy(out=st8f[:], in_=pf2[0:64, 0:8])
                ar2_in = dram.tile([64, 8], f32, tag="ar2i")
                ar2_out = dram.tile([64, 8], f32, tag="ar2o")
                nc.sync.dma_start(out=ar2_in[:], in_=st8f[:])
                nc.gpsimd.collective_compute(
                    "AllReduce", Alu.add, replica_groups=[list(range(N_CORES))],
                    ins=[ar2_in[:].opt()], outs=[ar2_out[:].opt()])
                nc.sync.dma_start(out=ar2_sb[:], in_=ar2_out[:])
                if debug:
                    nc.sync.dma_start(out=dbg['stats8'][:], in_=ar2_sb[:])

            # per-block scale/shift
            scs = []
            bnw = const.tile([64, 16], f32, tag="bnw")
            for blk_i, BL in enumerate((BLK_A, BLK_B) if do_post else ()):
                c0 = 4 * blk_i
                w0 = 8 * blk_i
                rN = 1.0 / BL['Ndp']
                nc.vector.tensor_add(out=bnw[:, w0:w0 + 1], in0=ar2_sb[:, c0:c0 + 1], in1=ar2_sb[:, c0 + 2:c0 + 3])
                nc.vector.tensor_scalar_mul(out=bnw[:, w0:w0 + 1], in0=bnw[:, w0:w0 + 1], scalar1=rN)  # mu
                nc.vector.tensor_add(out=bnw[:, w0 + 1:w0 + 2], in0=ar2_sb[:, c0 + 1:c0 + 2], in1=ar2_sb[:, c0 + 3:c0 + 4])
                nc.vector.tensor_scalar_mul(out=bnw[:, w0 + 1:w0 + 2], in0=bnw[:, w0 + 1:w0 + 2], scalar1=rN)  # E[g^2]
                nc.vector.tensor_mul(out=bnw[:, w0 + 2:w0 + 3], in0=bnw[:, w0:w0 + 1], in1=bnw[:, w0:w0 + 1])
                nc.vector.tensor_sub(out=bnw[:, w0 + 3:w0 + 4], in0=bnw[:, w0 + 1:w0 + 2], in1=bnw[:, w0 + 2:w0 + 3])  # var
                nc.scalar.activation(out=bnw[:, w0 + 4:w0 + 5], in_=bnw[:, w0 + 3:w0 + 4], func=Act.Sqrt,
                                     bias=eps_sb[:])
                nc.vector.reciprocal(out=bnw[:, w0 + 5:w0 + 6], in_=bnw[:, w0 + 4:w0 + 5])
                nc.vector.tensor_mul(out=bnw[:, w0 + 6:w0 + 7], in0=gamma_sb[:], in1=bnw[:, w0 + 5:w0 + 6])  # scale
                nc.vector.tensor_mul(out=bnw[:, w0 + 7:w0 + 8], in0=bnw[:, w0:w0 + 1], in1=bnw[:, w0 + 6:w0 + 7])
                sc128 = const.tile([128, 1], f32, tag=f"sc128_{blk_i}")
                sh128 = const.tile([128, 1], f32, tag=f"sh128_{blk_i}")
                nc.vector.tensor_copy(out=sc128[0:64, :], in_=bnw[:, w0 + 6:w0 + 7])
                nc.vector.tensor_copy(out=sc128[64:128, :], in_=bnw[:, w0 + 6:w0 + 7])
                nc.vector.tensor_sub(out=sh128[0:64, :], in0=beta_sb[:], in1=bnw[:, w0 + 7:w0 + 8])
                nc.vector.tensor_copy(out=sh128[64:128, :], in_=sh128[0:64, :])
                scs.append((sc128, sh128))

            # ---------------- bn apply -> u tiles ----------------
            u_tiles = {}
            for p in range(PAIRS if do_post else 0):
                for blk_i, BL in enumerate((BLK_A, BLK_B)):
                    EW, AW, PL = BL['EW'], BL['AW'], BL['PL']
                    tagb = f"b{blk_i}"
                    sc128, sh128 = scs[blk_i]
                    gt = g_tiles[(p, blk_i)]
                    bnv = tmp_p.tile([128, BLK_A["AW"]], f16, tag="t1")
                    bmin = tmp_p.tile([128, BLK_A["AW"]], f16, tag="t2")
                    bmax = tmp_p.tile([128, BLK_A["AW"]], f16, tag="t3")
                    nc.vector.tensor_scalar(out=bnv[:, 0:PL], in0=gt[:, 0:PL],
                                            scalar1=sc128[:], scalar2=sh128[:],
                                            op0=Alu.mult, op1=Alu.add)
                    nc.vector.tensor_scalar_min(out=bmin[:, 0:PL], in0=bnv[:, 0:PL], scalar1=0.0)
                    nc.vector.tensor_scalar_max(out=bmax[:, 0:PL], in0=bnv[:, 0:PL], scalar1=0.0)
                    nc.scalar.activation(out=bmin[:, 0:PL], in_=bmin[:, 0:PL], func=Act.Exp)
                    ut = gu_p.tile([128, AW], f16, tag="u" + tagb)
                    nc.vector.scalar_tensor_tensor(out=ut[:, 0:PL], in0=bmax[:, 0:PL],
                                                   scalar=-1.0, in1=bmin[:, 0:PL],
                                                   op0=Alu.add, op1=Alu.add)
                    u_tiles[(p, blk_i)] = ut
                    if debug and p == 0 and blk_i == 0:
                        nc.sync.dma_start(out=dbg['g_A'][:], in_=gt[:])
                        nc.sync.dma_start(out=dbg['u_A'][:], in_=ut[:])

            # ---------------- final assembly ----------------
            for p in range(PAIRS if do_post else 0):
                for ci in range(6144 // CH):
                    cs_u, ce_u = ci * CH, (ci + 1) * CH
                    if (p, ci) in stage_tiles:
                        stage = stage_tiles[(p, ci)]
                    else:
                        stage = stage_p.tile([128, CH], f32, tag="stage")
                        _emit_ops(p, ci, stage, 'copy')
                    _emit_ops(p, ci, stage, 'add')
                    nc.sync.dma_start(out=out_t[2 * p, :, cs_u:ce_u], in_=stage[0:64, :])
                    nc.scalar.dma_start(out=out_t[2 * p + 1, :, cs_u:ce_u], in_=stage[64:128, :])

    nc.finalize()
    return nc


# ---------------------------------------------------------------------------
# public entry
# ---------------------------------------------------------------------------

_cache = {}
_lock = threading.Lock()


def _get_program(debug=False):
    with _lock:
        key = bool(debug)
        if key not in _cache:
            _cache[key] = _build_program(debug=debug)
        return _cache[key]


def kernel(x, conv_v, conv_g, conv_b, bn_gamma, bn_beta, _debug=False, _trace=False):
    x = np.ascontiguousarray(np.asarray(x, dtype=np.float32))
    conv_v = np.asarray(conv_v, dtype=np.float32)
    conv_g = np.asarray(conv_g, dtype=np.float32)
    conv_b = np.asarray(conv_b, dtype=np.float32)
    bn_gamma = np.asarray(bn_gamma, dtype=np.float32)
    bn_beta = np.asarray(bn_beta, dtype=np.float32)

    nc = _get_program(debug=_debug)
    in_maps = []
    for ci in range(N_CORES):
        in_maps.append(dict(
            x=x[ci * BPC:(ci + 1) * BPC],
            conv_v=conv_v, conv_g=conv_g, conv_b=conv_b,
            bn_gamma=bn_gamma, bn_beta=bn_beta,
        ))
    res = run_bass_kernel_spmd(nc, in_maps, core_ids=list(range(N_CORES)),
                               trace=_trace)
    out = np.concatenate([res.results[ci]["out"] for ci in range(N_CORES)], axis=0)
    if _debug or _trace:
        return out, res
    return out



# revision 46
# speedup vs baseline: 1.0136x; 1.0136x over previous
"""Trainium2 Bass kernel for nn_Block_6579889898195 (ragged_sequence).

Self-contained: hardcodes shapes/sharding. Data-parallel over batch across 8
NeuronCores; the two global reductions (mean/std of x, batchnorm batch stats)
are AllReduce collectives.

Mathematical restructuring (validated against the reference in numpy):
  * the joint_explict interleave + dilated conv collapses into three dense
    stride-2 convs P/Q/Q' over the odd/even phases of x, with the 0.8/mean and
    0.8/std scalings folded into an affine recombination of P and Q
  * maxpool is computed from the even/odd split of the conv output
  * the dpadding gather is folded into weighted batchnorm sums on the compact
    grid plus a final composed gather
  * the output permutation (joint/sew_up/dpad composition) is precomputed on
    the host into ~32 strided access-pattern families
"""
import os
import sys
import threading

import numpy as np

for _p in ("/opt/trn_rl_repo", "/root/.axon_site/_ro/trn_rl_repo"):
    if os.path.isdir(_p) and _p not in sys.path:
        sys.path.insert(0, _p)

import concourse.bass as bass
import concourse.tile as tile
from concourse import bacc, mybir
from concourse.bass_utils import run_bass_kernel_spmd
from concourse.masks import make_identity

dt = mybir.dt
Alu = mybir.AluOpType
Act = mybir.ActivationFunctionType
Ax = mybir.AxisListType

N_CORES = 8
B, T, C = 64, 4096, 64
BPC = B // N_CORES          # batches per core
PAIRS = BPC // 2
KERNEL, STRIDE, DILATION, BN_EPS = 3, 2, 3, 1e-5
TH = T // 2                 # 2048, width of XN/PV tiles

# block geometry: (M, YL, PL, EW, dup_hi, Ndp, alloc_w, xn_off, xn_str, pv_off, pv_str, qp_off)
# A: conv over xn=xT[1::2], pv=xT[0::2]; rhs XN[2m+k], PV[2m+k], PV[2m+1+k]
# B: xn=xT[3::4]=XN[2i+1], pv=xT[0::4]=PV[2i]; rhs XN[4m+2k+1], PV[4m+2k], PV[4m+2k+2]
BLK_A = dict(M=1023, YL=3069, PL=1535, EW=1536, dup_hi=1026, Ndp=B * 2048, AW=1544,
             xn0=lambda k: k, xns=2, pv0=lambda k: k, pvs=2, qp0=lambda k: k + 1)
BLK_B = dict(M=511, YL=1533, PL=767, EW=768, dup_hi=514, Ndp=B * 1024, AW=776,
             xn0=lambda k: 2 * k + 1, xns=4, pv0=lambda k: 2 * k, pvs=4, qp0=lambda k: 2 * k + 2)

NTOT = B * T * C  # global element count of x


# ---------------------------------------------------------------------------
# static index plan (host side)
# ---------------------------------------------------------------------------

def _sew_up_indices(a_len, b_len, cur_layer, sp, st):
    idx = []
    cnt_a = cnt_b = 0
    while cnt_a < a_len:
        if sp:
            break
        if cnt_a == 0:
            pv = cnt_b
            for _ in range(st['skip_p']):
                idx.append(a_len + pv)
                cnt_b += 1
            if cur_layer % 2 != 0:
                idx.append(0)
                cnt_a += 1
            if cnt_a == 0:
                cnt_a = 1
            continue
        for _ in range(st['skip_n']):
            if cnt_b >= b_len:
                break
            idx.append(a_len + cnt_b)
            cnt_b += 1
        if not st['skip_t']:
            for _ in range(st['skip_s']):
                if cnt_b >= b_len:
                    break
                idx.append(a_len + cnt_b)
                cnt_b += 1
                st['skip_t'] = True
        else:
            for _j in range(st['skip_d']):
                for _i in range(st['skip_s']):
                    if cnt_b >= b_len:
                        break
                    idx.append(a_len + cnt_b)
                    cnt_b += 1
                if cnt_a >= a_len:
                    break
                idx.append(cnt_a)
                cnt_a += 1
                st['skip_t'] = False
            continue
        idx.append(cnt_a)
        cnt_a += 1
    idx += [a_len + j for j in range(cnt_b, b_len)]
    if sp:
        idx += list(range(cnt_a, a_len))
    st['skip_s'] += 1
    st['skip_n'] = 3 * st['skip_n'] + st['skip_s'] + st['skip_d'] * st['skip_s']
    if cur_layer % 2 != 0:
        st['skip_p'] += 1
    return np.asarray(idx, np.int64)


def _dpadding_indices(a_len, num_padding):
    if num_padding == 0:
        return np.arange(a_len, dtype=np.int64)
    skip_cnt = a_len // num_padding
    mult = None
    if skip_cnt == 0:
        mult = num_padding // a_len
        skip_cnt = 1
    entries = []
    rem = num_padding
    for i in range(a_len):
        if rem == 0:
            entries.append(list(range(i, a_len)))
            break
        if i % skip_cnt == 0:
            entries.append([i])
            rem -= 1
        if mult is not None:
            entries.extend([[i]] * mult)
        entries.append([i])
    return np.asarray([k for e in entries for k in e], np.int64)


def _build_final_map():
    st = dict(skip_p=1, skip_s=1, skip_d=2, skip_n=3, skip_t=False)
    si = _sew_up_indices(2048, 4096, 1, False, st)
    pi_A = _dpadding_indices(1535, 513)
    pi_B = _dpadding_indices(767, 257)
    U = len(si)
    kind = np.empty(U, np.int64)
    xi = np.empty(U, np.int64)
    ui = np.full(U, -1, np.int64)
    for u, s in enumerate(si):
        if s < 2048:
            t = int(s)
            if t % 2 == 0:
                pj = (4 * (t // 2)) // 2         # PV column
                kind[u] = 0 if pj % 2 == 0 else 3
                xi[u] = pj // 2                  # PVE/PVO column
            else:
                j = (t - 1) // 2
                kind[u] = 2
                xi[u] = j                        # XNo column (xn col 2j+1)
                ui[u] = pi_B[j]
        else:
            r = int(s) - 2048
            if r % 2 == 0:
                pj = r // 2
                kind[u] = 0 if pj % 2 == 0 else 3
                xi[u] = pj // 2
            else:
                j = (r - 1) // 2
                kind[u] = 1 if j % 2 == 0 else 4  # XNe / XNo column
                xi[u] = j // 2
                ui[u] = pi_A[j]
    return kind, xi, ui


def _plan_ops():
    kind, xi, ui = _build_final_map()
    U = len(kind)
    used = np.zeros(U, bool)
    ops = []
    for k in (0, 3, 1, 4, 2):
        usl = np.where(kind == k)[0].tolist()
        pos = set(usl)
        idx_of = {u: i for i, u in enumerate(usl)}
        for u0 in usl:
            if used[u0]:
                continue
            best = None
            cands = []
            i0 = idx_of[u0]
            for nxt in usl[i0 + 1:i0 + 40]:
                if not used[nxt]:
                    cands.append(nxt - u0)
                if len(cands) >= 30:
                    break
            for du in cands:
                dx0 = xi[u0 + du] - xi[u0] if (u0 + du) in pos else None
                if dx0 is None or dx0 <= 0:
                    continue
                if k in (1, 4, 2) and ui[u0 + du] - ui[u0] <= 0:
                    continue
                cnt = 1
                u = u0
                while True:
                    un = u + du
                    if un >= U or un not in pos or used[un]:
                        break
                    if xi[un] - xi[u] != dx0:
                        break
                    if k in (1, 4, 2) and (ui[un] - ui[u] != ui[u0 + du] - ui[u0]):
                        break
                    u = un
                    cnt += 1
                if best is None or cnt > best[1]:
                    best = (du, cnt)
            du, cnt = best if best else (1, 1)
            if cnt == 1:
                ops.append((k, u0, 1, 1, int(xi[u0]), 0, int(ui[u0]), 0))
                used[u0] = True
                continue
            dx = int(xi[u0 + du] - xi[u0])
            dui = int(ui[u0 + du] - ui[u0]) if k in (1, 4, 2) else 0
            for t in range(cnt):
                used[u0 + t * du] = True
            ops.append((k, int(u0), int(du), int(cnt), int(xi[u0]), dx, int(ui[u0]), dui))
    assert used.all()
    return ops


def _sl(tileap, start, step, count):
    """Strided free-dim slice [start : start+(count-1)*step+1 : step]."""
    if count == 1 or step == 1:
        return tileap[:, start:start + count]
    assert step > 0
    return tileap[:, start:start + (count - 1) * step + 1:step]


# ---------------------------------------------------------------------------
# program builder
# ---------------------------------------------------------------------------

def _build_program(debug=False):
    KLEVEL = int(os.environ.get("KLEVEL", "3"))
    KPARTS = int(os.environ.get("KPARTS", "7"))
    nc = bacc.Bacc(num_devices=N_CORES)
    f32, f32r, f16 = dt.float32, dt.float32r, dt.float16

    x_t = nc.dram_tensor("x", [BPC, T, C], f32, kind="ExternalInput")
    v_t = nc.dram_tensor("conv_v", [C, C, KERNEL], f32, kind="ExternalInput")
    g_t = nc.dram_tensor("conv_g", [C], f32, kind="ExternalInput")
    b_t = nc.dram_tensor("conv_b", [C], f32, kind="ExternalInput")
    gam_t = nc.dram_tensor("bn_gamma", [C], f32, kind="ExternalInput")
    bet_t = nc.dram_tensor("bn_beta", [C], f32, kind="ExternalInput")
    out_t = nc.dram_tensor("out", [BPC, C, 6144], f32, kind="ExternalOutput")
    dbg = {}
    if debug:
        dbg['g_A'] = nc.dram_tensor("dbg_g_A", [128, 1544], dt.float16, kind="ExternalOutput")
        dbg['u_A'] = nc.dram_tensor("dbg_u_A", [128, 1544], dt.float16, kind="ExternalOutput")
        dbg['stats8'] = nc.dram_tensor("dbg_stats8", [64, 8], f32, kind="ExternalOutput")
        dbg['ac'] = nc.dram_tensor("dbg_ac", [1, 2], f32, kind="ExternalOutput")
        dbg['yA'] = nc.dram_tensor("dbg_yA", [128, 1544], dt.float16, kind="ExternalOutput")
        dbg['yoA'] = nc.dram_tensor("dbg_yoA", [128, 1544], dt.float16, kind="ExternalOutput")
        dbg['poolA'] = nc.dram_tensor("dbg_poolA", [128, 1544], dt.float16, kind="ExternalOutput")
        dbg['zA'] = nc.dram_tensor("dbg_zA", [128, 1544], dt.float16, kind="ExternalOutput")

    plan = _plan_ops()
    CH = 3072

    with tile.TileContext(nc) as tc:
        import contextlib
        ctx = contextlib.ExitStack()
        with ctx:
            const = ctx.enter_context(tc.tile_pool(name="const", bufs=1))
            xload_p = ctx.enter_context(tc.tile_pool(name="xload", bufs=2))
            xn_p = ctx.enter_context(tc.tile_pool(name="xn", bufs=PAIRS))
            pv_p = ctx.enter_context(tc.tile_pool(name="pv", bufs=PAIRS))
            y_p = ctx.enter_context(tc.tile_pool(name="y", bufs=1))
            pool_p = ctx.enter_context(tc.tile_pool(name="pool", bufs=1))
            tmp_p = ctx.enter_context(tc.tile_pool(name="tmp", bufs=2))
            gu_p = ctx.enter_context(tc.tile_pool(name="gu", bufs=PAIRS))
            pv0_p = ctx.enter_context(tc.tile_pool(name="pv0", bufs=1))
            qsb_p = ctx.enter_context(tc.tile_pool(name="qsb", bufs=4))
            stat_p = ctx.enter_context(tc.tile_pool(name="stat", bufs=2 * PAIRS))
            stage_p = ctx.enter_context(tc.tile_pool(name="stage", bufs=3))
            tp_ps = ctx.enter_context(tc.tile_pool(name="tp_ps", bufs=2, space="PSUM"))
            conv_ps = ctx.enter_context(tc.tile_pool(name="conv_ps", bufs=5, space="PSUM"))
            misc_ps = ctx.enter_context(tc.tile_pool(name="misc_ps", bufs=1, space="PSUM"))
            dram = ctx.enter_context(tc.tile_pool(name="dram", bufs=8, space="DRAM"))

            # ---------------- constants & parameters ----------------
            ident = const.tile([128, 128], f32, tag="ident")
            make_identity(nc, ident[:])
            ones = const.tile([128, 1], f32, tag="ones")
            nc.vector.memset(ones[:], 1.0)
            ii = const.tile([128, 64], f32, tag="ii")
            make_identity(nc, ii[0:64, :])
            make_identity(nc, ii[64:128, :])

            bias128 = const.tile([128, 1], f32, tag="bias128")
            nc.sync.dma_start(out=bias128[0:64, :], in_=b_t[:])
            nc.sync.dma_start(out=bias128[64:128, :], in_=b_t[:])
            gamma_sb = const.tile([64, 1], f32, tag="gamma")
            nc.sync.dma_start(out=gamma_sb[:], in_=gam_t[:])
            beta_sb = const.tile([64, 1], f32, tag="beta")
            nc.sync.dma_start(out=beta_sb[:], in_=bet_t[:])
            eps_sb = const.tile([64, 1], f32, tag="eps")
            nc.vector.memset(eps_sb[:], BN_EPS)

            # ---------------- weight prep ----------------
            do_wp = KPARTS & 4
            vt = const.tile([64, 192], f32, tag="vt")
            nc.sync.dma_start(out=vt[:], in_=v_t[:])
            gg = const.tile([64, 1], f32, tag="gg")
            nc.sync.dma_start(out=gg[:], in_=g_t[:])
            dumw = const.tile([64, 192], f32, tag="dumw")
            nrm2 = const.tile([64, 4], f32, tag="nrm2")
            lhsP = const.tile([128, 3, 128], f32, tag="lhsP")
            lhsPr = const.tile([128, 3, 128], f16, tag="lhsPr")
            lhsQ = const.tile([128, 3, 128], f16, tag="lhsQ")
            wsc = const.tile([64, 192], f32, tag="wsc")
            if do_wp:
                nc.vector.tensor_mul(out=dumw[:], in0=vt[:], in1=vt[:])
                nc.vector.tensor_reduce(out=nrm2[:, 0:1], in_=dumw[:], axis=Ax.X, op=Alu.add)
                nc.scalar.activation(out=nrm2[:, 1:2], in_=nrm2[:, 0:1], func=Act.Sqrt)
                nc.vector.reciprocal(out=nrm2[:, 2:3], in_=nrm2[:, 1:2])
                nc.vector.tensor_mul(out=nrm2[:, 3:4], in0=gg[:], in1=nrm2[:, 2:3])
                nc.vector.tensor_scalar_mul(out=wsc[:], in0=vt[:], scalar1=nrm2[:, 3:4])
                KWP = int(os.environ.get("KWP", "3"))
                nc.vector.memset(lhsP[:], 0.0)
                if KWP >= 2:
                    pw = misc_ps.tile([128, 512], f32, tag="misc")
                    for k in range(3):
                        nc.tensor.transpose(pw[0:64, 64 * k:64 * k + 64], wsc[:, k:192:3], ident[0:64, 0:64])
                    if KWP >= 3:
                        for k in range(3):
                            nc.vector.tensor_copy(out=lhsP[0:64, k, 0:64], in_=pw[0:64, 64 * k:64 * k + 64])
                            nc.vector.tensor_copy(out=lhsP[64:128, k, 64:128], in_=pw[0:64, 64 * k:64 * k + 64])
                nc.vector.tensor_copy(out=lhsPr[:], in_=lhsP[:])
                nc.scalar.mul(out=lhsQ[:], in_=lhsP[:], mul=0.2)
            else:
                nc.vector.memset(lhsP[:], 0.0)
                nc.vector.tensor_copy(out=lhsPr[:], in_=lhsP[:])
                nc.vector.tensor_copy(out=lhsQ[:], in_=lhsP[:])

            # ---------------- loads + x-stats ----------------
            # Per pair of batches, DMA three strided t-streams directly from
            # HBM (xn = odd t, pve = t%4==0, pvo = t%4==2), packing the two
            # batches' channels side by side in the free dim so one [128,128]
            # PE transpose handles both batches at once.
            xbst = const.tile([128, 4 * BPC, 6], f32, tag="xbst")
            lload = []
            for p in range(PAIRS):
                lxne = xload_p.tile([128, 8, 128], f32, tag="lxne")
                lxno = xload_p.tile([128, 8, 128], f32, tag="lxno")
                lpve = xload_p.tile([128, 8, 128], f32, tag="lpve")
                lpvo = xload_p.tile([128, 8, 128], f32, tag="lpvo")
                # one DMA per stream per batch; issue queues spread across
                # engines so descriptor generation doesn't serialize on Sync
                for dlt in range(2):
                    b = 2 * p + dlt
                    cs = slice(64 * dlt, 64 * dlt + 64)
                    nc.sync.dma_start(
                        out=lxne[:, :, cs],
                        in_=x_t[b].rearrange("(n p q) c -> q p n c", p=128, q=4)[1])
                    nc.sync.dma_start(
                        out=lxno[:, :, cs],
                        in_=x_t[b].rearrange("(n p q) c -> q p n c", p=128, q=4)[3])
                    nc.scalar.dma_start(
                        out=lpve[:, :, cs],
                        in_=x_t[b].rearrange("(n p q) c -> q p n c", p=128, q=4)[0])
                    nc.gpsimd.dma_start(
                        out=lpvo[:, :, cs],
                        in_=x_t[b].rearrange("(n p q) c -> q p n c", p=128, q=4)[2])
                if KPARTS & 1:
                    chk = 0
                    for src, nch in ((lxne, 2), (lxno, 2), (lpve, 2), (lpvo, 2)):
                        f = src[:].rearrange("p n c -> p (n c)")
                        for c0 in range(nch):
                            nc.vector.bn_stats(out=xbst[:, 8 * p + chk, :],
                                               in_=f[:, 512 * c0:512 * (c0 + 1)])
                            chk += 1
                lload.append((lxne, lxno, lpve, lpvo))
            if not (KPARTS & 1):
                nc.vector.memset(xbst[:], 0.5)
            xmv = const.tile([128, 8], f32, tag="xmv")
            nc.vector.bn_aggr(out=xmv[:, 0:2], in_=xbst[:])
            # sums per partition: s = mean*16384 ; ss = (var + mean^2)*16384
            nc.vector.tensor_mul(out=xmv[:, 2:3], in0=xmv[:, 0:1], in1=xmv[:, 0:1])
            nc.vector.tensor_add(out=xmv[:, 3:4], in0=xmv[:, 1:2], in1=xmv[:, 2:3])
            xsums = const.tile([128, 2], f32, tag="xsums")
            nc.vector.tensor_scalar_mul(out=xsums[:, 0:1], in0=xmv[:, 0:1], scalar1=float(T * C * BPC // 128))
            nc.vector.tensor_scalar_mul(out=xsums[:, 1:2], in0=xmv[:, 3:4], scalar1=float(T * C * BPC // 128))
            pf1 = misc_ps.tile([128, 512], f32, tag="misc")
            nc.tensor.matmul(pf1[0:1, 0:2], ones[:], xsums[:], start=True, stop=True)
            s12 = const.tile([1, 2], f32, tag="s12")
            nc.vector.tensor_copy(out=s12[:], in_=pf1[0:1, 0:2])
            ar1_in = dram.tile([1, 2], f32, tag="ar1i")
            ar1_out = dram.tile([1, 2], f32, tag="ar1o")
            nc.sync.dma_start(out=ar1_in[:], in_=s12[:])
            nc.gpsimd.collective_compute(
                "AllReduce", Alu.add, replica_groups=[list(range(N_CORES))],
                ins=[ar1_in[:].opt()], outs=[ar1_out[:].opt()])


            # ---------------- transposes -> XN / PV per pair ----------------
            if KLEVEL < 1:
                raise SystemExit
            XNE, XNO, XNE1, PVE, PVO, PV00, PV02 = [], [], [], [], [], [], []
            f32r = dt.float32r
            for p in range(PAIRS):
                xne = xn_p.tile([128, TH // 2], f32, tag="xne")
                xno = xn_p.tile([128, TH // 2], f32, tag="xno")
                pve = pv_p.tile([128, TH // 2], f16, tag="pve")
                pvo = pv_p.tile([128, TH // 2], f16, tag="pvo")
                lxne, lxno, lpve, lpvo = lload[p]
                di = p  # stagger drain-engine rotation across pairs
                for (lt, nblk, dst) in ((lxne, 8, xne), (lxno, 8, xno),
                                        (lpve, 8, pve), (lpvo, 8, pvo)):
                    if not (KPARTS & 2):
                        break
                    for s in range(nblk // 4):
                        ptp = tp_ps.tile([128, 512], f32, tag="tp")
                        for q in range(4):
                            nc.tensor.transpose(ptp[:, 128 * q:128 * (q + 1)],
                                                lt[:, 4 * s + q, :], ident[:])
                        dsl = dst[:, 512 * s:512 * (s + 1)]
                        # gpsimd cannot read PSUM; alternate scalar/vector
                        if di % 2 == 0:
                            nc.scalar.copy(out=dsl, in_=ptp[:])
                        else:
                            nc.vector.tensor_copy(out=dsl, in_=ptp[:])
                        di += 1
                pv00 = pv0_p.tile([128, TH // 4], f16, tag="pv00")
                pv02 = pv0_p.tile([128, TH // 4], f16, tag="pv02")
                # shifted copy of XNe so the k=2 P-tap is even-aligned for f32r
                xne1 = pv0_p.tile([128, TH // 2], f32, tag="xne1")
                if KPARTS & 2:
                    nc.scalar.copy(out=pv00[:], in_=pve[:, 0:1024:2])
                    nc.vector.tensor_copy(out=pv02[:], in_=pve[:, 1:1024:2])
                    nc.vector.tensor_copy(out=xne1[:, 0:1023].bitcast(f32),
                                          in_=xne[:, 1:1024].bitcast(f32))
                    nc.vector.memset(xne1[:, 1023:1024].bitcast(f32), 0.0)
                XNE.append(xne)
                XNO.append(xno)
                XNE1.append(xne1)
                PVE.append(pve)
                PVO.append(pvo)
                PV00.append(pv00)
                PV02.append(pv02)

            # ---------------- assembly helpers + pair-0 passthrough ----------
            # kind 0/3 copies depend only on PVE/PVO; emitting pair 0's early
            # fills engine idle time during the AR1 collective.
            eng_acc = {'v': 8000.0, 's': 14000.0, 'g': 0.0}

            def _op_cost(e, n):
                return {'v': n * 1.04 + 105.0, 's': n * 0.83 + 124.0,
                        'g': n * 1.05 + 131.0}[e]

            def _pick(cand, n):
                e = min(cand, key=lambda e: eng_acc[e] + _op_cost(e, n))
                eng_acc[e] += _op_cost(e, n)
                return e

            def _emit_ops(p, ci, stage, which):
                cs_u, ce_u = ci * CH, (ci + 1) * CH
                for (k, u0, du, cnt, x0, dx, ui0, dui) in plan:
                    if (k in (0, 3)) != (which == 'copy'):
                        continue
                    t0 = max(0, -(-(cs_u - u0) // du))
                    t1 = min(cnt - 1, (ce_u - 1 - u0) // du)
                    if t1 < t0:
                        continue
                    n = t1 - t0 + 1
                    us = u0 + t0 * du - cs_u
                    xs = x0 + t0 * dx
                    if k in (0, 3):
                        src = PVE[p][:] if k == 0 else PVO[p][:]
                        e = _pick(('s', 'g', 'v'), n)
                        fn = {'s': nc.scalar.copy, 'g': nc.gpsimd.tensor_copy,
                              'v': nc.vector.tensor_copy}[e]
                        fn(out=_sl(stage, us, du, n), in_=_sl(src, xs, dx, n))
                    else:
                        uu = ui0 + t0 * dui
                        ut = u_tiles[(p, 0 if k in (1, 4) else 1)]
                        srcx = (XNE[p] if k == 1 else XNO[p])[:].bitcast(f32)
                        e = _pick(('v', 'g'), n)
                        eng = nc.vector if e == 'v' else nc.gpsimd
                        eng.tensor_add(out=_sl(stage, us, du, n),
                                       in0=_sl(srcx, xs, dx, n),
                                       in1=_sl(ut, uu, dui, n))

            stage_tiles = {}
            if KLEVEL >= 3:
                for (pp, ci) in ((0, 0), (0, 1), (1, 0)):
                    stage = stage_p.tile([128, CH], f32, tag="stage")
                    stage_tiles[(pp, ci)] = stage
                    _emit_ops(pp, ci, stage, 'copy')

            # a/c chain emitted late so its AR1 wait does not block queues
            gs = const.tile([1, 12], f32, tag="gs")
            nc.sync.dma_start(out=gs[:, 0:2], in_=ar1_out[:])
            # a = 0.8*N/S1 ; c = 0.8/std, std = sqrt((S2 - S1^2/N)/(N-1))
            nc.vector.reciprocal(out=gs[:, 2:3], in_=gs[:, 0:1])
            nc.vector.tensor_scalar_mul(out=gs[:, 3:4], in0=gs[:, 2:3], scalar1=float(0.8 * NTOT))  # a
            nc.vector.tensor_scalar_mul(out=gs[:, 4:5], in0=gs[:, 0:1], scalar1=float(1.0 / NTOT))
            nc.vector.tensor_mul(out=gs[:, 5:6], in0=gs[:, 0:1], in1=gs[:, 4:5])
            nc.vector.tensor_sub(out=gs[:, 6:7], in0=gs[:, 1:2], in1=gs[:, 5:6])
            nc.vector.tensor_scalar_mul(out=gs[:, 7:8], in0=gs[:, 6:7], scalar1=float(1.0 / (NTOT - 1)))
            nc.scalar.activation(out=gs[:, 8:9], in_=gs[:, 7:8], func=Act.Sqrt)
            nc.vector.reciprocal(out=gs[:, 9:10], in_=gs[:, 8:9])
            nc.vector.tensor_scalar_mul(out=gs[:, 10:11], in0=gs[:, 9:10], scalar1=0.8)  # c
            ac_pack = const.tile([1, 2], f32, tag="acp")
            nc.vector.tensor_copy(out=ac_pack[:, 0:1], in_=gs[:, 3:4])
            nc.vector.tensor_copy(out=ac_pack[:, 1:2], in_=gs[:, 10:11])
            a128 = const.tile([128, 1], f32, tag="a128")
            c128 = const.tile([128, 1], f32, tag="c128")
            if int(os.environ.get("KNOBCAST", "0")):
                nc.vector.memset(a128[:], 1.0)
                nc.vector.memset(c128[:], 1.0)
            else:
                nc.gpsimd.partition_broadcast(a128[:], ac_pack[0:1, 0:1])
                nc.gpsimd.partition_broadcast(c128[:], ac_pack[0:1, 1:2])
            if debug:
                nc.sync.dma_start(out=dbg['ac'][:], in_=ac_pack[:])

            # ---------------- conv + pointwise per pair/block ----------------
            do_conv = KLEVEL >= 2
            g_tiles = {}   # (p, blk) -> g tile (f16)
            st_tiles = {}  # (p, blk) -> [128,4] stats
            for p in range(PAIRS if do_conv else 0):
                qtiles = {0: (PVE[p], PVO[p]), 1: (PV00[p], PV02[p])}
                for blk_i, BL in enumerate((BLK_A, BLK_B)):
                    M, PL, EW, AW = BL['M'], BL['PL'], BL['EW'], BL['AW']
                    tagb = f"b{blk_i}"
                    ye = y_p.tile([128, BLK_A["AW"]], f16, tag="ye")
                    yo = y_p.tile([128, BLK_A["AW"]], f16, tag="yo")
                    nc.gpsimd.memset(yo[:, 0:1], float("-inf"))
                    nc.gpsimd.memset(yo[:, PL:PL + 2], float("-inf"))
                    nc.gpsimd.memset(ye[:, PL:PL + 1], 0.0)
                    halves = [(0, 512, 512), (512, 512, 511)] if blk_i == 0 else [(0, 512, 511)]
                    for (m0, mw, realw) in halves:
                        psP = conv_ps.tile([128, 512], f32, tag="conv")
                        psQ = conv_ps.tile([128, 512], f32, tag="conv")
                        psQp = conv_ps.tile([128, 512], f32, tag="conv")
                        if blk_i == 0:
                            # P_A must be full fp32 (a ~ 5e4 amplifies its
                            # error); f32r turned out to be reduced-precision.
                            # Dense even taps on XNe/XNo/XNe1.
                            taps = (XNE[p], XNO[p], XNE1[p])
                            for k in range(3):
                                nc.tensor.matmul(psP[:, 0:512], lhsP[:, k, :],
                                                 taps[k][:, m0:m0 + 512].bitcast(f32),
                                                 start=(k == 0), stop=(k == 2))
                        else:
                            # P_B: fp32 with stride-2 taps on XNo
                            xnof = XNO[p][:].bitcast(f32)
                            for k in range(3):
                                nc.tensor.matmul(psP[:, 0:realw], lhsP[:, k, :],
                                                 _sl(xnof, k, 2, realw),
                                                 start=(k == 0), stop=(k == 2))
                        qe, qo = qtiles[blk_i]
                        # fp16 taps allow arbitrary column alignment:
                        # psQ  = R_even = 0.2*(W0 qe[m] + W1 qo[m] + W2 qe[m+1])
                        # psQp = R_odd  =      W0 qo[m] + W1 qe[m+1] + W2 qo[m+1]
                        for ps, lt, taps in ((psQ, lhsQ, ((qe, 0), (qo, 0), (qe, 1))),
                                             (psQp, lhsPr, ((qo, 0), (qe, 1), (qo, 1)))):
                            for k, (qt, off) in enumerate(taps):
                                nc.tensor.matmul(ps[:, 0:realw], lt[:, k, :],
                                                 qt[:, m0 + off:m0 + off + realw],
                                                 start=(k == 0), stop=(k == 2))
                        # Drain conv psums to SBUF immediately (AR1-independent)
                        # so PE never stalls on the collective: the a/c-scaled
                        # STT reads these SBUF copies later.
                        qsb = qsb_p.tile([128, 512], f16, tag="qsb")
                        nc.scalar.copy(out=qsb[:, 0:realw], in_=psQ[:, 0:realw])
                        psb = qsb_p.tile([128, 512], f32, tag="psb")
                        nc.scalar.copy(out=psb[:, 0:realw], in_=psP[:, 0:realw])
                        # scatter phases into ye/yo (q = parity of m); ph2 first
                        # (no a/c dependency) so psQp frees early too
                        for ph in (2, 0, 1):
                            for q in range(2):
                                ms = m0 if m0 % 2 == q else m0 + 1
                                if ms >= m0 + realw:
                                    continue
                                cnt = (m0 + realw - ms + 1) // 2
                                l0 = 3 * ms + ph
                                if l0 % 2 == 0:
                                    ytile, ycol = ye, l0 // 2
                                else:
                                    ytile, ycol = yo, (l0 - 1) // 2 + 1
                                pscol = ms - m0
                                if ph == 2:
                                    nc.scalar.copy(out=_sl(ytile, ycol, 3, cnt),
                                                   in_=_sl(psQp, pscol, 2, cnt))
                                else:
                                    sc = a128 if ph == 0 else c128
                                    nc.vector.scalar_tensor_tensor(
                                        out=_sl(ytile, ycol, 3, cnt),
                                        in0=_sl(psb, pscol, 2, cnt), scalar=sc[:],
                                        in1=_sl(qsb, pscol, 2, cnt),
                                        op0=Alu.mult, op1=Alu.add)
                    if debug and p == 0 and blk_i == 0:
                        nc.sync.dma_start(out=dbg['yA'][:], in_=ye[:])
                        nc.sync.dma_start(out=dbg['yoA'][:], in_=yo[:])
                    # ---- pool ----
                    pool = pool_p.tile([128, BLK_A["AW"]], f16, tag="pool")
                    nc.vector.tensor_max(out=pool[:, 0:EW], in0=yo[:, 0:EW], in1=ye[:, 0:EW])
                    nc.vector.tensor_max(out=pool[:, 0:EW], in0=pool[:, 0:EW], in1=yo[:, 1:EW + 1])
                    # ---- v = pool + bias ; chain to g ----
                    vmin = tmp_p.tile([128, BLK_A["AW"]], f16, tag="t1")
                    vmax = tmp_p.tile([128, BLK_A["AW"]], f16, tag="t2")
                    ee = tmp_p.tile([128, BLK_A["AW"]], f16, tag="t3")
                    nc.vector.tensor_scalar(out=vmin[:, 0:EW], in0=pool[:, 0:EW],
                                            scalar1=bias128[:], scalar2=0.0,
                                            op0=Alu.add, op1=Alu.min)
                    nc.vector.tensor_scalar(out=vmax[:, 0:EW], in0=pool[:, 0:EW],
                                            scalar1=bias128[:], scalar2=0.0,
                                            op0=Alu.add, op1=Alu.max)
                    nc.scalar.activation(out=ee[:, 0:EW], in_=vmin[:, 0:EW], func=Act.Exp)
                    # z = vmax - 1 + e  (reuse vmax tile)
                    nc.vector.scalar_tensor_tensor(out=vmax[:, 0:EW], in0=vmax[:, 0:EW],
                                                   scalar=-1.0, in1=ee[:, 0:EW],
                                                   op0=Alu.add, op1=Alu.add)
                    # zsq (reuse vmin)
                    if debug and p == 0 and blk_i == 0:
                        nc.sync.dma_start(out=dbg['poolA'][:], in_=pool[:])
                        nc.sync.dma_start(out=dbg['zA'][:], in_=vmax[:])
                    nc.vector.tensor_mul(out=vmin[:, 0:EW], in0=vmax[:, 0:EW], in1=vmax[:, 0:EW])
                    gt = gu_p.tile([128, AW], f16, tag="g" + tagb)
                    st4 = stat_p.tile([128, 4], f32, tag="st4")
                    nc.scalar.activation(out=gt[:, 0:PL], in_=vmin[:, 0:PL], func=Act.Exp,
                                         scale=-0.5, accum_out=st4[:, 0:1])
                    nc.scalar.activation(out=ee[:, 0:PL], in_=gt[:, 0:PL], func=Act.Square,
                                         accum_out=st4[:, 1:2])
                    dcnt = BL['dup_hi'] // 2
                    nc.scalar.activation(out=vmax[:, 0:dcnt], in_=_sl(gt, 0, 2, dcnt),
                                         func=Act.Identity, accum_out=st4[:, 2:3])
                    nc.scalar.activation(out=vmin[:, 0:dcnt], in_=_sl(gt, 0, 2, dcnt),
                                         func=Act.Square, accum_out=st4[:, 3:4])
                    g_tiles[(p, blk_i)] = gt
                    st_tiles[(p, blk_i)] = st4

            # ---------------- AR2: batchnorm stats ----------------
            do_post = KLEVEL >= 3 and do_conv
            stats8 = const.tile([128, 8], f32, tag="stats8")
            for blk_i in range(2 if do_post else 0):
                cs = 4 * blk_i
                nc.vector.tensor_copy(out=stats8[:, cs:cs + 4], in_=st_tiles[(0, blk_i)][:])
                for p in range(1, PAIRS):
                    nc.vector.tensor_add(out=stats8[:, cs:cs + 4], in0=stats8[:, cs:cs + 4],
                                         in1=st_tiles[(p, blk_i)][:])
            pf2 = misc_ps.tile([128, 512], f32, tag="misc")
            if do_post:
                nc.tensor.matmul(pf2[0:64, 0:8], ii[:], stats8[:], start=True, stop=True)
            st8f = const.tile([64, 8], f32, tag="st8f")
            ar2_sb = const.tile([64, 8], f32, tag="ar2sb")
            if do_post:
                nc.vector.tensor_copy(out=st8f[:], in_=pf2[0:64, 0:8])
                ar2_in = dram.tile([64, 8], f32, tag="ar2i")
                ar2_out = dram.tile([64, 8], f32, tag="ar2o")
                nc.sync.dma_start(out=ar2_in[:], in_=st8f[:])
                nc.gpsimd.collective_compute(
                    "AllReduce", Alu.add, replica_groups=[list(range(N_CORES))],
                    ins=[ar2_in[:].opt()], outs=[ar2_out[:].opt()])
                nc.sync.dma_start(out=ar2_sb[:], in_=ar2_out[:])
                if debug:
                    nc.sync.dma_start(out=dbg['stats8'][:], in_=ar2_sb[:])

            # per-block scale/shift
            scs = []
            bnw = const.tile([64, 16], f32, tag="bnw")
            for blk_i, BL in enumerate((BLK_A, BLK_B) if do_post else ()):
                c0 = 4 * blk_i
                w0 = 8 * blk_i
                rN = 1.0 / BL['Ndp']
                nc.vector.tensor_add(out=bnw[:, w0:w0 + 1], in0=ar2_sb[:, c0:c0 + 1], in1=ar2_sb[:, c0 + 2:c0 + 3])
                nc.vector.tensor_scalar_mul(out=bnw[:, w0:w0 + 1], in0=bnw[:, w0:w0 + 1], scalar1=rN)  # mu
                nc.vector.tensor_add(out=bnw[:, w0 + 1:w0 + 2], in0=ar2_sb[:, c0 + 1:c0 + 2], in1=ar2_sb[:, c0 + 3:c0 + 4])
                nc.vector.tensor_scalar_mul(out=bnw[:, w0 + 1:w0 + 2], in0=bnw[:, w0 + 1:w0 + 2], scalar1=rN)  # E[g^2]
                nc.vector.tensor_mul(out=bnw[:, w0 + 2:w0 + 3], in0=bnw[:, w0:w0 + 1], in1=bnw[:, w0:w0 + 1])
                nc.vector.tensor_sub(out=bnw[:, w0 + 3:w0 + 4], in0=bnw[:, w0 + 1:w0 + 2], in1=bnw[:, w0 + 2:w0 + 3])  # var
                nc.scalar.activation(out=bnw[:, w0 + 4:w0 + 5], in_=bnw[:, w0 + 3:w0 + 4], func=Act.Sqrt,
                                     bias=eps_sb[:])
                nc.vector.reciprocal(out=bnw[:, w0 + 5:w0 + 6], in_=bnw[:, w0 + 4:w0 + 5])
                nc.vector.tensor_mul(out=bnw[:, w0 + 6:w0 + 7], in0=gamma_sb[:], in1=bnw[:, w0 + 5:w0 + 6])  # scale
                nc.vector.tensor_mul(out=bnw[:, w0 + 7:w0 + 8], in0=bnw[:, w0:w0 + 1], in1=bnw[:, w0 + 6:w0 + 7])
                sc128 = const.tile([128, 1], f32, tag=f"sc128_{blk_i}")
                sh128 = const.tile([128, 1], f32, tag=f"sh128_{blk_i}")
                nc.vector.tensor_copy(out=sc128[0:64, :], in_=bnw[:, w0 + 6:w0 + 7])
                nc.vector.tensor_copy(out=sc128[64:128, :], in_=bnw[:, w0 + 6:w0 + 7])
                nc.vector.tensor_sub(out=sh128[0:64, :], in0=beta_sb[:], in1=bnw[:, w0 + 7:w0 + 8])
                nc.vector.tensor_copy(out=sh128[64:128, :], in_=sh128[0:64, :])
                scs.append((sc128, sh128))

            # ---------------- bn apply -> u tiles ----------------
            u_tiles = {}
            for p in range(PAIRS if do_post else 0):
                for blk_i, BL in enumerate((BLK_A, BLK_B)):
                    EW, AW, PL = BL['EW'], BL['AW'], BL['PL']
                    tagb = f"b{blk_i}"
                    sc128, sh128 = scs[blk_i]
                    gt = g_tiles[(p, blk_i)]
                    bnv = tmp_p.tile([128, BLK_A["AW"]], f16, tag="t1")
                    bmin = tmp_p.tile([128, BLK_A["AW"]], f16, tag="t2")
                    bmax = tmp_p.tile([128, BLK_A["AW"]], f16, tag="t3")
                    nc.vector.tensor_scalar(out=bnv[:, 0:PL], in0=gt[:, 0:PL],
                                            scalar1=sc128[:], scalar2=sh128[:],
                                            op0=Alu.mult, op1=Alu.add)
                    nc.vector.tensor_scalar_min(out=bmin[:, 0:PL], in0=bnv[:, 0:PL], scalar1=0.0)
                    nc.vector.tensor_scalar_max(out=bmax[:, 0:PL], in0=bnv[:, 0:PL], scalar1=0.0)
                    nc.scalar.activation(out=bmin[:, 0:PL], in_=bmin[:, 0:PL], func=Act.Exp)
                    ut = gu_p.tile([128, AW], f16, tag="u" + tagb)
                    nc.vector.scalar_tensor_tensor(out=ut[:, 0:PL], in0=bmax[:, 0:PL],
                                                   scalar=-1.0, in1=bmin[:, 0:PL],
                                                   op0=Alu.add, op1=Alu.add)
                    u_tiles[(p, blk_i)] = ut
                    if debug and p == 0 and blk_i == 0:
                        nc.sync.dma_start(out=dbg['g_A'][:], in_=gt[:])
                        nc.sync.dma_start(out=dbg['u_A'][:], in_=ut[:])

            # ---------------- final assembly ----------------
            for p in range(PAIRS if do_post else 0):
                for ci in range(6144 // CH):
                    cs_u, ce_u = ci * CH, (ci + 1) * CH
                    if (p, ci) in stage_tiles:
                        stage = stage_tiles[(p, ci)]
                    else:
                        stage = stage_p.tile([128, CH], f32, tag="stage")
                        _emit_ops(p, ci, stage, 'copy')
                    _emit_ops(p, ci, stage, 'add')
                    nc.sync.dma_start(out=out_t[2 * p, :, cs_u:ce_u], in_=stage[0:64, :])
                    nc.sync.dma_start(out=out_t[2 * p + 1, :, cs_u:ce_u], in_=stage[64:128, :])

    nc.finalize()
    return nc


# ---------------------------------------------------------------------------
# public entry
# ---------------------------------------------------------------------------

_cache = {}
_lock = threading.Lock()


def _get_program(debug=False):
    with _lock:
        key = bool(debug)
        if key not in _cache:
            _cache[key] = _build_program(debug=debug)
        return _cache[key]


def kernel(x, conv_v, conv_g, conv_b, bn_gamma, bn_beta, _debug=False, _trace=False):
    x = np.ascontiguousarray(np.asarray(x, dtype=np.float32))
    conv_v = np.asarray(conv_v, dtype=np.float32)
    conv_g = np.asarray(conv_g, dtype=np.float32)
    conv_b = np.asarray(conv_b, dtype=np.float32)
    bn_gamma = np.asarray(bn_gamma, dtype=np.float32)
    bn_beta = np.asarray(bn_beta, dtype=np.float32)

    nc = _get_program(debug=_debug)
    in_maps = []
    for ci in range(N_CORES):
        in_maps.append(dict(
            x=x[ci * BPC:(ci + 1) * BPC],
            conv_v=conv_v, conv_g=conv_g, conv_b=conv_b,
            bn_gamma=bn_gamma, bn_beta=bn_beta,
        ))
    res = run_bass_kernel_spmd(nc, in_maps, core_ids=list(range(N_CORES)),
                               trace=_trace)
    out = np.concatenate([res.results[ci]["out"] for ci in range(N_CORES)], axis=0)
    if _debug or _trace:
        return out, res
    return out



# revision 47
# speedup vs baseline: 1.0485x; 1.0344x over previous
"""Trainium2 Bass kernel for nn_Block_6579889898195 (ragged_sequence).

Self-contained: hardcodes shapes/sharding. Data-parallel over batch across 8
NeuronCores; the two global reductions (mean/std of x, batchnorm batch stats)
are AllReduce collectives.

Mathematical restructuring (validated against the reference in numpy):
  * the joint_explict interleave + dilated conv collapses into three dense
    stride-2 convs P/Q/Q' over the odd/even phases of x, with the 0.8/mean and
    0.8/std scalings folded into an affine recombination of P and Q
  * maxpool is computed from the even/odd split of the conv output
  * the dpadding gather is folded into weighted batchnorm sums on the compact
    grid plus a final composed gather
  * the output permutation (joint/sew_up/dpad composition) is precomputed on
    the host into ~32 strided access-pattern families
"""
import os
import sys
import threading

import numpy as np

for _p in ("/opt/trn_rl_repo", "/root/.axon_site/_ro/trn_rl_repo"):
    if os.path.isdir(_p) and _p not in sys.path:
        sys.path.insert(0, _p)

import concourse.bass as bass
import concourse.tile as tile
from concourse import bacc, mybir
from concourse.bass_utils import run_bass_kernel_spmd
from concourse.masks import make_identity

dt = mybir.dt
Alu = mybir.AluOpType
Act = mybir.ActivationFunctionType
Ax = mybir.AxisListType

N_CORES = 8
B, T, C = 64, 4096, 64
BPC = B // N_CORES          # batches per core
PAIRS = BPC // 2
KERNEL, STRIDE, DILATION, BN_EPS = 3, 2, 3, 1e-5
TH = T // 2                 # 2048, width of XN/PV tiles

# block geometry: (M, YL, PL, EW, dup_hi, Ndp, alloc_w, xn_off, xn_str, pv_off, pv_str, qp_off)
# A: conv over xn=xT[1::2], pv=xT[0::2]; rhs XN[2m+k], PV[2m+k], PV[2m+1+k]
# B: xn=xT[3::4]=XN[2i+1], pv=xT[0::4]=PV[2i]; rhs XN[4m+2k+1], PV[4m+2k], PV[4m+2k+2]
BLK_A = dict(M=1023, YL=3069, PL=1535, EW=1536, dup_hi=1026, Ndp=B * 2048, AW=1544,
             xn0=lambda k: k, xns=2, pv0=lambda k: k, pvs=2, qp0=lambda k: k + 1)
BLK_B = dict(M=511, YL=1533, PL=767, EW=768, dup_hi=514, Ndp=B * 1024, AW=776,
             xn0=lambda k: 2 * k + 1, xns=4, pv0=lambda k: 2 * k, pvs=4, qp0=lambda k: 2 * k + 2)

NTOT = B * T * C  # global element count of x


# ---------------------------------------------------------------------------
# static index plan (host side)
# ---------------------------------------------------------------------------

def _sew_up_indices(a_len, b_len, cur_layer, sp, st):
    idx = []
    cnt_a = cnt_b = 0
    while cnt_a < a_len:
        if sp:
            break
        if cnt_a == 0:
            pv = cnt_b
            for _ in range(st['skip_p']):
                idx.append(a_len + pv)
                cnt_b += 1
            if cur_layer % 2 != 0:
                idx.append(0)
                cnt_a += 1
            if cnt_a == 0:
                cnt_a = 1
            continue
        for _ in range(st['skip_n']):
            if cnt_b >= b_len:
                break
            idx.append(a_len + cnt_b)
            cnt_b += 1
        if not st['skip_t']:
            for _ in range(st['skip_s']):
                if cnt_b >= b_len:
                    break
                idx.append(a_len + cnt_b)
                cnt_b += 1
                st['skip_t'] = True
        else:
            for _j in range(st['skip_d']):
                for _i in range(st['skip_s']):
                    if cnt_b >= b_len:
                        break
                    idx.append(a_len + cnt_b)
                    cnt_b += 1
                if cnt_a >= a_len:
                    break
                idx.append(cnt_a)
                cnt_a += 1
                st['skip_t'] = False
            continue
        idx.append(cnt_a)
        cnt_a += 1
    idx += [a_len + j for j in range(cnt_b, b_len)]
    if sp:
        idx += list(range(cnt_a, a_len))
    st['skip_s'] += 1
    st['skip_n'] = 3 * st['skip_n'] + st['skip_s'] + st['skip_d'] * st['skip_s']
    if cur_layer % 2 != 0:
        st['skip_p'] += 1
    return np.asarray(idx, np.int64)


def _dpadding_indices(a_len, num_padding):
    if num_padding == 0:
        return np.arange(a_len, dtype=np.int64)
    skip_cnt = a_len // num_padding
    mult = None
    if skip_cnt == 0:
        mult = num_padding // a_len
        skip_cnt = 1
    entries = []
    rem = num_padding
    for i in range(a_len):
        if rem == 0:
            entries.append(list(range(i, a_len)))
            break
        if i % skip_cnt == 0:
            entries.append([i])
            rem -= 1
        if mult is not None:
            entries.extend([[i]] * mult)
        entries.append([i])
    return np.asarray([k for e in entries for k in e], np.int64)


def _build_final_map():
    st = dict(skip_p=1, skip_s=1, skip_d=2, skip_n=3, skip_t=False)
    si = _sew_up_indices(2048, 4096, 1, False, st)
    pi_A = _dpadding_indices(1535, 513)
    pi_B = _dpadding_indices(767, 257)
    U = len(si)
    kind = np.empty(U, np.int64)
    xi = np.empty(U, np.int64)
    ui = np.full(U, -1, np.int64)
    for u, s in enumerate(si):
        if s < 2048:
            t = int(s)
            if t % 2 == 0:
                pj = (4 * (t // 2)) // 2         # PV column
                kind[u] = 0 if pj % 2 == 0 else 3
                xi[u] = pj // 2                  # PVE/PVO column
            else:
                j = (t - 1) // 2
                kind[u] = 2
                xi[u] = j                        # XNo column (xn col 2j+1)
                ui[u] = pi_B[j]
        else:
            r = int(s) - 2048
            if r % 2 == 0:
                pj = r // 2
                kind[u] = 0 if pj % 2 == 0 else 3
                xi[u] = pj // 2
            else:
                j = (r - 1) // 2
                kind[u] = 1 if j % 2 == 0 else 4  # XNe / XNo column
                xi[u] = j // 2
                ui[u] = pi_A[j]
    return kind, xi, ui


def _plan_ops():
    kind, xi, ui = _build_final_map()
    U = len(kind)
    used = np.zeros(U, bool)
    ops = []
    for k in (0, 3, 1, 4, 2):
        usl = np.where(kind == k)[0].tolist()
        pos = set(usl)
        idx_of = {u: i for i, u in enumerate(usl)}
        for u0 in usl:
            if used[u0]:
                continue
            best = None
            cands = []
            i0 = idx_of[u0]
            for nxt in usl[i0 + 1:i0 + 40]:
                if not used[nxt]:
                    cands.append(nxt - u0)
                if len(cands) >= 30:
                    break
            for du in cands:
                dx0 = xi[u0 + du] - xi[u0] if (u0 + du) in pos else None
                if dx0 is None or dx0 <= 0:
                    continue
                if k in (1, 4, 2) and ui[u0 + du] - ui[u0] <= 0:
                    continue
                cnt = 1
                u = u0
                while True:
                    un = u + du
                    if un >= U or un not in pos or used[un]:
                        break
                    if xi[un] - xi[u] != dx0:
                        break
                    if k in (1, 4, 2) and (ui[un] - ui[u] != ui[u0 + du] - ui[u0]):
                        break
                    u = un
                    cnt += 1
                if best is None or cnt > best[1]:
                    best = (du, cnt)
            du, cnt = best if best else (1, 1)
            if cnt == 1:
                ops.append((k, u0, 1, 1, int(xi[u0]), 0, int(ui[u0]), 0))
                used[u0] = True
                continue
            dx = int(xi[u0 + du] - xi[u0])
            dui = int(ui[u0 + du] - ui[u0]) if k in (1, 4, 2) else 0
            for t in range(cnt):
                used[u0 + t * du] = True
            ops.append((k, int(u0), int(du), int(cnt), int(xi[u0]), dx, int(ui[u0]), dui))
    assert used.all()
    return ops


def _sl(tileap, start, step, count):
    """Strided free-dim slice [start : start+(count-1)*step+1 : step]."""
    if count == 1 or step == 1:
        return tileap[:, start:start + count]
    assert step > 0
    return tileap[:, start:start + (count - 1) * step + 1:step]


# ---------------------------------------------------------------------------
# program builder
# ---------------------------------------------------------------------------

def _build_program(debug=False):
    KLEVEL = int(os.environ.get("KLEVEL", "3"))
    KPARTS = int(os.environ.get("KPARTS", "7"))
    nc = bacc.Bacc(num_devices=N_CORES)
    f32, f32r, f16 = dt.float32, dt.float32r, dt.float16

    x_t = nc.dram_tensor("x", [BPC, T, C], f32, kind="ExternalInput")
    v_t = nc.dram_tensor("conv_v", [C, C, KERNEL], f32, kind="ExternalInput")
    g_t = nc.dram_tensor("conv_g", [C], f32, kind="ExternalInput")
    b_t = nc.dram_tensor("conv_b", [C], f32, kind="ExternalInput")
    gam_t = nc.dram_tensor("bn_gamma", [C], f32, kind="ExternalInput")
    bet_t = nc.dram_tensor("bn_beta", [C], f32, kind="ExternalInput")
    out_t = nc.dram_tensor("out", [BPC, C, 6144], f32, kind="ExternalOutput")
    dbg = {}
    if debug:
        dbg['g_A'] = nc.dram_tensor("dbg_g_A", [128, 1544], dt.float16, kind="ExternalOutput")
        dbg['u_A'] = nc.dram_tensor("dbg_u_A", [128, 1544], dt.float16, kind="ExternalOutput")
        dbg['stats8'] = nc.dram_tensor("dbg_stats8", [64, 8], f32, kind="ExternalOutput")
        dbg['ac'] = nc.dram_tensor("dbg_ac", [1, 2], f32, kind="ExternalOutput")
        dbg['yA'] = nc.dram_tensor("dbg_yA", [128, 1544], dt.float16, kind="ExternalOutput")
        dbg['yoA'] = nc.dram_tensor("dbg_yoA", [128, 1544], dt.float16, kind="ExternalOutput")
        dbg['poolA'] = nc.dram_tensor("dbg_poolA", [128, 1544], dt.float16, kind="ExternalOutput")
        dbg['zA'] = nc.dram_tensor("dbg_zA", [128, 1544], dt.float16, kind="ExternalOutput")

    plan = _plan_ops()
    CH = 3072

    with tile.TileContext(nc) as tc:
        import contextlib
        ctx = contextlib.ExitStack()
        with ctx:
            const = ctx.enter_context(tc.tile_pool(name="const", bufs=1))
            xload_p = ctx.enter_context(tc.tile_pool(name="xload", bufs=2))
            xn_p = ctx.enter_context(tc.tile_pool(name="xn", bufs=PAIRS))
            pv_p = ctx.enter_context(tc.tile_pool(name="pv", bufs=PAIRS))
            y_p = ctx.enter_context(tc.tile_pool(name="y", bufs=1))
            pool_p = ctx.enter_context(tc.tile_pool(name="pool", bufs=1))
            tmp_p = ctx.enter_context(tc.tile_pool(name="tmp", bufs=2))
            gu_p = ctx.enter_context(tc.tile_pool(name="gu", bufs=PAIRS))
            pv0_p = ctx.enter_context(tc.tile_pool(name="pv0", bufs=1))
            qsb_p = ctx.enter_context(tc.tile_pool(name="qsb", bufs=4))
            stat_p = ctx.enter_context(tc.tile_pool(name="stat", bufs=2 * PAIRS))
            stage_p = ctx.enter_context(tc.tile_pool(name="stage", bufs=3))
            tp_ps = ctx.enter_context(tc.tile_pool(name="tp_ps", bufs=2, space="PSUM"))
            conv_ps = ctx.enter_context(tc.tile_pool(name="conv_ps", bufs=5, space="PSUM"))
            misc_ps = ctx.enter_context(tc.tile_pool(name="misc_ps", bufs=1, space="PSUM"))
            dram = ctx.enter_context(tc.tile_pool(name="dram", bufs=8, space="DRAM"))

            # ---------------- constants & parameters ----------------
            ident = const.tile([128, 128], f32, tag="ident")
            make_identity(nc, ident[:])
            ones = const.tile([128, 1], f32, tag="ones")
            nc.vector.memset(ones[:], 1.0)
            ii = const.tile([128, 64], f32, tag="ii")
            make_identity(nc, ii[0:64, :])
            make_identity(nc, ii[64:128, :])

            bias128 = const.tile([128, 1], f32, tag="bias128")
            nc.sync.dma_start(out=bias128[0:64, :], in_=b_t[:])
            nc.sync.dma_start(out=bias128[64:128, :], in_=b_t[:])
            gamma_sb = const.tile([64, 1], f32, tag="gamma")
            nc.sync.dma_start(out=gamma_sb[:], in_=gam_t[:])
            beta_sb = const.tile([64, 1], f32, tag="beta")
            nc.sync.dma_start(out=beta_sb[:], in_=bet_t[:])
            eps_sb = const.tile([64, 1], f32, tag="eps")
            nc.vector.memset(eps_sb[:], BN_EPS)

            # ---------------- weight prep ----------------
            do_wp = KPARTS & 4
            vt = const.tile([64, 192], f32, tag="vt")
            nc.sync.dma_start(out=vt[:], in_=v_t[:])
            gg = const.tile([64, 1], f32, tag="gg")
            nc.sync.dma_start(out=gg[:], in_=g_t[:])
            dumw = const.tile([64, 192], f32, tag="dumw")
            nrm2 = const.tile([64, 4], f32, tag="nrm2")
            lhsP = const.tile([128, 3, 128], f32, tag="lhsP")
            lhsPr = const.tile([128, 3, 128], f16, tag="lhsPr")
            lhsQ = const.tile([128, 3, 128], f16, tag="lhsQ")
            wsc = const.tile([64, 192], f32, tag="wsc")
            if do_wp:
                nc.vector.tensor_mul(out=dumw[:], in0=vt[:], in1=vt[:])
                nc.vector.tensor_reduce(out=nrm2[:, 0:1], in_=dumw[:], axis=Ax.X, op=Alu.add)
                nc.scalar.activation(out=nrm2[:, 1:2], in_=nrm2[:, 0:1], func=Act.Sqrt)
                nc.vector.reciprocal(out=nrm2[:, 2:3], in_=nrm2[:, 1:2])
                nc.vector.tensor_mul(out=nrm2[:, 3:4], in0=gg[:], in1=nrm2[:, 2:3])
                nc.vector.tensor_scalar_mul(out=wsc[:], in0=vt[:], scalar1=nrm2[:, 3:4])
                KWP = int(os.environ.get("KWP", "3"))
                nc.vector.memset(lhsP[:], 0.0)
                if KWP >= 2:
                    pw = misc_ps.tile([128, 512], f32, tag="misc")
                    for k in range(3):
                        nc.tensor.transpose(pw[0:64, 64 * k:64 * k + 64], wsc[:, k:192:3], ident[0:64, 0:64])
                    if KWP >= 3:
                        for k in range(3):
                            nc.vector.tensor_copy(out=lhsP[0:64, k, 0:64], in_=pw[0:64, 64 * k:64 * k + 64])
                            nc.vector.tensor_copy(out=lhsP[64:128, k, 64:128], in_=pw[0:64, 64 * k:64 * k + 64])
                nc.vector.tensor_copy(out=lhsPr[:], in_=lhsP[:])
                nc.scalar.mul(out=lhsQ[:], in_=lhsP[:], mul=0.2)
            else:
                nc.vector.memset(lhsP[:], 0.0)
                nc.vector.tensor_copy(out=lhsPr[:], in_=lhsP[:])
                nc.vector.tensor_copy(out=lhsQ[:], in_=lhsP[:])

            # ---------------- loads + x-stats ----------------
            # Per pair of batches, DMA three strided t-streams directly from
            # HBM (xn = odd t, pve = t%4==0, pvo = t%4==2), packing the two
            # batches' channels side by side in the free dim so one [128,128]
            # PE transpose handles both batches at once.
            xbst = const.tile([128, 4 * BPC, 6], f32, tag="xbst")
            lload = []
            for p in range(PAIRS):
                lxne = xload_p.tile([128, 8, 128], f32, tag="lxne")
                lxno = xload_p.tile([128, 8, 128], f32, tag="lxno")
                lpve = xload_p.tile([128, 8, 128], f32, tag="lpve")
                lpvo = xload_p.tile([128, 8, 128], f32, tag="lpvo")
                # one DMA per stream per batch; issue queues spread across
                # engines so descriptor generation doesn't serialize on Sync
                for dlt in range(2):
                    b = 2 * p + dlt
                    cs = slice(64 * dlt, 64 * dlt + 64)
                    nc.sync.dma_start(
                        out=lxne[:, :, cs],
                        in_=x_t[b].rearrange("(n p q) c -> q p n c", p=128, q=4)[1])
                    nc.sync.dma_start(
                        out=lxno[:, :, cs],
                        in_=x_t[b].rearrange("(n p q) c -> q p n c", p=128, q=4)[3])
                    nc.scalar.dma_start(
                        out=lpve[:, :, cs],
                        in_=x_t[b].rearrange("(n p q) c -> q p n c", p=128, q=4)[0])
                    nc.gpsimd.dma_start(
                        out=lpvo[:, :, cs],
                        in_=x_t[b].rearrange("(n p q) c -> q p n c", p=128, q=4)[2])
                if KPARTS & 1:
                    chk = 0
                    for src, nch in ((lxne, 2), (lxno, 2), (lpve, 2), (lpvo, 2)):
                        f = src[:].rearrange("p n c -> p (n c)")
                        for c0 in range(nch):
                            nc.vector.bn_stats(out=xbst[:, 8 * p + chk, :],
                                               in_=f[:, 512 * c0:512 * (c0 + 1)])
                            chk += 1
                lload.append((lxne, lxno, lpve, lpvo))
            if not (KPARTS & 1):
                nc.vector.memset(xbst[:], 0.5)
            xmv = const.tile([128, 8], f32, tag="xmv")
            nc.vector.bn_aggr(out=xmv[:, 0:2], in_=xbst[:])
            # sums per partition: s = mean*16384 ; ss = (var + mean^2)*16384
            nc.vector.tensor_mul(out=xmv[:, 2:3], in0=xmv[:, 0:1], in1=xmv[:, 0:1])
            nc.vector.tensor_add(out=xmv[:, 3:4], in0=xmv[:, 1:2], in1=xmv[:, 2:3])
            xsums = const.tile([128, 2], f32, tag="xsums")
            nc.vector.tensor_scalar_mul(out=xsums[:, 0:1], in0=xmv[:, 0:1], scalar1=float(T * C * BPC // 128))
            nc.vector.tensor_scalar_mul(out=xsums[:, 1:2], in0=xmv[:, 3:4], scalar1=float(T * C * BPC // 128))
            pf1 = misc_ps.tile([128, 512], f32, tag="misc")
            nc.tensor.matmul(pf1[0:1, 0:2], ones[:], xsums[:], start=True, stop=True)
            s12 = const.tile([1, 2], f32, tag="s12")
            nc.vector.tensor_copy(out=s12[:], in_=pf1[0:1, 0:2])
            ar1_in = dram.tile([1, 2], f32, tag="ar1i")
            ar1_out = dram.tile([1, 2], f32, tag="ar1o")
            nc.sync.dma_start(out=ar1_in[:], in_=s12[:])
            nc.gpsimd.collective_compute(
                "AllReduce", Alu.add, replica_groups=[list(range(N_CORES))],
                ins=[ar1_in[:].opt()], outs=[ar1_out[:].opt()])


            # ---------------- transposes -> XN / PV per pair ----------------
            if KLEVEL < 1:
                raise SystemExit
            XNE, XNO, XNE1, PVE, PVO, PV00, PV02 = [], [], [], [], [], [], []
            f32r = dt.float32r
            for p in range(PAIRS):
                xne = xn_p.tile([128, TH // 2], f32, tag="xne")
                xno = xn_p.tile([128, TH // 2], f32, tag="xno")
                pve = pv_p.tile([128, TH // 2], f16, tag="pve")
                pvo = pv_p.tile([128, TH // 2], f16, tag="pvo")
                lxne, lxno, lpve, lpvo = lload[p]
                di = p  # stagger drain-engine rotation across pairs
                for (lt, nblk, dst) in ((lxne, 8, xne), (lxno, 8, xno),
                                        (lpve, 8, pve), (lpvo, 8, pvo)):
                    if not (KPARTS & 2):
                        break
                    for s in range(nblk // 4):
                        ptp = tp_ps.tile([128, 512], f32, tag="tp")
                        for q in range(4):
                            nc.tensor.transpose(ptp[:, 128 * q:128 * (q + 1)],
                                                lt[:, 4 * s + q, :], ident[:])
                        dsl = dst[:, 512 * s:512 * (s + 1)]
                        # gpsimd cannot read PSUM; 2/3 scalar, 1/3 vector
                        if di % 3 != 2:
                            nc.scalar.copy(out=dsl, in_=ptp[:])
                        else:
                            nc.vector.tensor_copy(out=dsl, in_=ptp[:])
                        di += 1
                pv00 = pv0_p.tile([128, TH // 4], f16, tag="pv00")
                pv02 = pv0_p.tile([128, TH // 4], f16, tag="pv02")
                # shifted copy of XNe so the k=2 P-tap is even-aligned for f32r
                xne1 = pv0_p.tile([128, TH // 2], f32, tag="xne1")
                if KPARTS & 2:
                    nc.scalar.copy(out=pv00[:], in_=pve[:, 0:1024:2])
                    nc.vector.tensor_copy(out=pv02[:], in_=pve[:, 1:1024:2])
                    nc.vector.tensor_copy(out=xne1[:, 0:1023].bitcast(f32),
                                          in_=xne[:, 1:1024].bitcast(f32))
                    nc.vector.memset(xne1[:, 1023:1024].bitcast(f32), 0.0)
                XNE.append(xne)
                XNO.append(xno)
                XNE1.append(xne1)
                PVE.append(pve)
                PVO.append(pvo)
                PV00.append(pv00)
                PV02.append(pv02)

            # ---------------- assembly helpers + pair-0 passthrough ----------
            # kind 0/3 copies depend only on PVE/PVO; emitting pair 0's early
            # fills engine idle time during the AR1 collective.
            eng_acc = {'v': 8000.0, 's': 14000.0, 'g': 0.0}

            def _op_cost(e, n):
                return {'v': n * 1.04 + 105.0, 's': n * 0.83 + 124.0,
                        'g': n * 1.05 + 131.0}[e]

            def _pick(cand, n):
                e = min(cand, key=lambda e: eng_acc[e] + _op_cost(e, n))
                eng_acc[e] += _op_cost(e, n)
                return e

            def _emit_ops(p, ci, stage, which):
                cs_u, ce_u = ci * CH, (ci + 1) * CH
                for (k, u0, du, cnt, x0, dx, ui0, dui) in plan:
                    if (k in (0, 3)) != (which == 'copy'):
                        continue
                    t0 = max(0, -(-(cs_u - u0) // du))
                    t1 = min(cnt - 1, (ce_u - 1 - u0) // du)
                    if t1 < t0:
                        continue
                    n = t1 - t0 + 1
                    us = u0 + t0 * du - cs_u
                    xs = x0 + t0 * dx
                    if k in (0, 3):
                        src = PVE[p][:] if k == 0 else PVO[p][:]
                        e = _pick(('s', 'g', 'v'), n)
                        fn = {'s': nc.scalar.copy, 'g': nc.gpsimd.tensor_copy,
                              'v': nc.vector.tensor_copy}[e]
                        fn(out=_sl(stage, us, du, n), in_=_sl(src, xs, dx, n))
                    else:
                        uu = ui0 + t0 * dui
                        ut = u_tiles[(p, 0 if k in (1, 4) else 1)]
                        srcx = (XNE[p] if k == 1 else XNO[p])[:].bitcast(f32)
                        e = _pick(('v', 'g'), n)
                        eng = nc.vector if e == 'v' else nc.gpsimd
                        eng.tensor_add(out=_sl(stage, us, du, n),
                                       in0=_sl(srcx, xs, dx, n),
                                       in1=_sl(ut, uu, dui, n))

            stage_tiles = {}
            if KLEVEL >= 3:
                for (pp, ci) in ((0, 0), (0, 1), (1, 0)):
                    stage = stage_p.tile([128, CH], f32, tag="stage")
                    stage_tiles[(pp, ci)] = stage
                    _emit_ops(pp, ci, stage, 'copy')

            # a/c chain emitted late so its AR1 wait does not block queues
            gs = const.tile([1, 12], f32, tag="gs")
            nc.sync.dma_start(out=gs[:, 0:2], in_=ar1_out[:])
            # a = 0.8*N/S1 ; c = 0.8/std, std = sqrt((S2 - S1^2/N)/(N-1))
            nc.vector.reciprocal(out=gs[:, 2:3], in_=gs[:, 0:1])
            nc.vector.tensor_scalar_mul(out=gs[:, 3:4], in0=gs[:, 2:3], scalar1=float(0.8 * NTOT))  # a
            nc.vector.tensor_scalar_mul(out=gs[:, 4:5], in0=gs[:, 0:1], scalar1=float(1.0 / NTOT))
            nc.vector.tensor_mul(out=gs[:, 5:6], in0=gs[:, 0:1], in1=gs[:, 4:5])
            nc.vector.tensor_sub(out=gs[:, 6:7], in0=gs[:, 1:2], in1=gs[:, 5:6])
            nc.vector.tensor_scalar_mul(out=gs[:, 7:8], in0=gs[:, 6:7], scalar1=float(1.0 / (NTOT - 1)))
            nc.scalar.activation(out=gs[:, 8:9], in_=gs[:, 7:8], func=Act.Sqrt)
            nc.vector.reciprocal(out=gs[:, 9:10], in_=gs[:, 8:9])
            nc.vector.tensor_scalar_mul(out=gs[:, 10:11], in0=gs[:, 9:10], scalar1=0.8)  # c
            ac_pack = const.tile([1, 2], f32, tag="acp")
            nc.vector.tensor_copy(out=ac_pack[:, 0:1], in_=gs[:, 3:4])
            nc.vector.tensor_copy(out=ac_pack[:, 1:2], in_=gs[:, 10:11])
            a128 = const.tile([128, 1], f32, tag="a128")
            c128 = const.tile([128, 1], f32, tag="c128")
            if int(os.environ.get("KNOBCAST", "0")):
                nc.vector.memset(a128[:], 1.0)
                nc.vector.memset(c128[:], 1.0)
            else:
                nc.gpsimd.partition_broadcast(a128[:], ac_pack[0:1, 0:1])
                nc.gpsimd.partition_broadcast(c128[:], ac_pack[0:1, 1:2])
            if debug:
                nc.sync.dma_start(out=dbg['ac'][:], in_=ac_pack[:])

            # ---------------- conv + pointwise per pair/block ----------------
            do_conv = KLEVEL >= 2
            g_tiles = {}   # (p, blk) -> g tile (f16)
            st_tiles = {}  # (p, blk) -> [128,4] stats
            for p in range(PAIRS if do_conv else 0):
                qtiles = {0: (PVE[p], PVO[p]), 1: (PV00[p], PV02[p])}
                for blk_i, BL in enumerate((BLK_A, BLK_B)):
                    M, PL, EW, AW = BL['M'], BL['PL'], BL['EW'], BL['AW']
                    tagb = f"b{blk_i}"
                    ye = y_p.tile([128, BLK_A["AW"]], f16, tag="ye")
                    yo = y_p.tile([128, BLK_A["AW"]], f16, tag="yo")
                    nc.gpsimd.memset(yo[:, 0:1], float("-inf"))
                    nc.gpsimd.memset(yo[:, PL:PL + 2], float("-inf"))
                    nc.gpsimd.memset(ye[:, PL:PL + 1], 0.0)
                    halves = [(0, 512, 512), (512, 512, 511)] if blk_i == 0 else [(0, 512, 511)]
                    for (m0, mw, realw) in halves:
                        psP = conv_ps.tile([128, 512], f32, tag="conv")
                        psQ = conv_ps.tile([128, 512], f32, tag="conv")
                        psQp = conv_ps.tile([128, 512], f32, tag="conv")
                        if blk_i == 0:
                            # P_A must be full fp32 (a ~ 5e4 amplifies its
                            # error); f32r turned out to be reduced-precision.
                            # Dense even taps on XNe/XNo/XNe1.
                            taps = (XNE[p], XNO[p], XNE1[p])
                            for k in range(3):
                                nc.tensor.matmul(psP[:, 0:512], lhsP[:, k, :],
                                                 taps[k][:, m0:m0 + 512].bitcast(f32),
                                                 start=(k == 0), stop=(k == 2))
                        else:
                            # P_B: fp32 with stride-2 taps on XNo
                            xnof = XNO[p][:].bitcast(f32)
                            for k in range(3):
                                nc.tensor.matmul(psP[:, 0:realw], lhsP[:, k, :],
                                                 _sl(xnof, k, 2, realw),
                                                 start=(k == 0), stop=(k == 2))
                        qe, qo = qtiles[blk_i]
                        # fp16 taps allow arbitrary column alignment:
                        # psQ  = R_even = 0.2*(W0 qe[m] + W1 qo[m] + W2 qe[m+1])
                        # psQp = R_odd  =      W0 qo[m] + W1 qe[m+1] + W2 qo[m+1]
                        for ps, lt, taps in ((psQ, lhsQ, ((qe, 0), (qo, 0), (qe, 1))),
                                             (psQp, lhsPr, ((qo, 0), (qe, 1), (qo, 1)))):
                            for k, (qt, off) in enumerate(taps):
                                nc.tensor.matmul(ps[:, 0:realw], lt[:, k, :],
                                                 qt[:, m0 + off:m0 + off + realw],
                                                 start=(k == 0), stop=(k == 2))
                        # Drain conv psums to SBUF immediately (AR1-independent)
                        # so PE never stalls on the collective: the a/c-scaled
                        # STT reads these SBUF copies later.
                        qsb = qsb_p.tile([128, 512], f16, tag="qsb")
                        nc.scalar.copy(out=qsb[:, 0:realw], in_=psQ[:, 0:realw])
                        psb = qsb_p.tile([128, 512], f32, tag="psb")
                        nc.scalar.copy(out=psb[:, 0:realw], in_=psP[:, 0:realw])
                        # scatter phases into ye/yo (q = parity of m); ph2 first
                        # (no a/c dependency) so psQp frees early too
                        for ph in (2, 0, 1):
                            for q in range(2):
                                ms = m0 if m0 % 2 == q else m0 + 1
                                if ms >= m0 + realw:
                                    continue
                                cnt = (m0 + realw - ms + 1) // 2
                                l0 = 3 * ms + ph
                                if l0 % 2 == 0:
                                    ytile, ycol = ye, l0 // 2
                                else:
                                    ytile, ycol = yo, (l0 - 1) // 2 + 1
                                pscol = ms - m0
                                if ph == 2:
                                    nc.scalar.copy(out=_sl(ytile, ycol, 3, cnt),
                                                   in_=_sl(psQp, pscol, 2, cnt))
                                else:
                                    sc = a128 if ph == 0 else c128
                                    nc.vector.scalar_tensor_tensor(
                                        out=_sl(ytile, ycol, 3, cnt),
                                        in0=_sl(psb, pscol, 2, cnt), scalar=sc[:],
                                        in1=_sl(qsb, pscol, 2, cnt),
                                        op0=Alu.mult, op1=Alu.add)
                    if debug and p == 0 and blk_i == 0:
                        nc.sync.dma_start(out=dbg['yA'][:], in_=ye[:])
                        nc.sync.dma_start(out=dbg['yoA'][:], in_=yo[:])
                    # ---- pool ----
                    pool = pool_p.tile([128, BLK_A["AW"]], f16, tag="pool")
                    nc.vector.tensor_max(out=pool[:, 0:EW], in0=yo[:, 0:EW], in1=ye[:, 0:EW])
                    nc.vector.tensor_max(out=pool[:, 0:EW], in0=pool[:, 0:EW], in1=yo[:, 1:EW + 1])
                    # ---- v = pool + bias ; chain to g ----
                    vmin = tmp_p.tile([128, BLK_A["AW"]], f16, tag="t1")
                    vmax = tmp_p.tile([128, BLK_A["AW"]], f16, tag="t2")
                    ee = tmp_p.tile([128, BLK_A["AW"]], f16, tag="t3")
                    nc.vector.tensor_scalar(out=vmin[:, 0:EW], in0=pool[:, 0:EW],
                                            scalar1=bias128[:], scalar2=0.0,
                                            op0=Alu.add, op1=Alu.min)
                    nc.vector.tensor_scalar(out=vmax[:, 0:EW], in0=pool[:, 0:EW],
                                            scalar1=bias128[:], scalar2=0.0,
                                            op0=Alu.add, op1=Alu.max)
                    nc.scalar.activation(out=ee[:, 0:EW], in_=vmin[:, 0:EW], func=Act.Exp)
                    # z = vmax - 1 + e  (reuse vmax tile)
                    nc.vector.scalar_tensor_tensor(out=vmax[:, 0:EW], in0=vmax[:, 0:EW],
                                                   scalar=-1.0, in1=ee[:, 0:EW],
                                                   op0=Alu.add, op1=Alu.add)
                    # zsq (reuse vmin)
                    if debug and p == 0 and blk_i == 0:
                        nc.sync.dma_start(out=dbg['poolA'][:], in_=pool[:])
                        nc.sync.dma_start(out=dbg['zA'][:], in_=vmax[:])
                    nc.vector.tensor_mul(out=vmin[:, 0:EW], in0=vmax[:, 0:EW], in1=vmax[:, 0:EW])
                    gt = gu_p.tile([128, AW], f16, tag="g" + tagb)
                    st4 = stat_p.tile([128, 4], f32, tag="st4")
                    nc.scalar.activation(out=gt[:, 0:PL], in_=vmin[:, 0:PL], func=Act.Exp,
                                         scale=-0.5, accum_out=st4[:, 0:1])
                    nc.scalar.activation(out=ee[:, 0:PL], in_=gt[:, 0:PL], func=Act.Square,
                                         accum_out=st4[:, 1:2])
                    dcnt = BL['dup_hi'] // 2
                    nc.scalar.activation(out=vmax[:, 0:dcnt], in_=_sl(gt, 0, 2, dcnt),
                                         func=Act.Identity, accum_out=st4[:, 2:3])
                    nc.scalar.activation(out=vmin[:, 0:dcnt], in_=_sl(gt, 0, 2, dcnt),
                                         func=Act.Square, accum_out=st4[:, 3:4])
                    g_tiles[(p, blk_i)] = gt
                    st_tiles[(p, blk_i)] = st4

            # ---------------- AR2: batchnorm stats ----------------
            do_post = KLEVEL >= 3 and do_conv
            stats8 = const.tile([128, 8], f32, tag="stats8")
            for blk_i in range(2 if do_post else 0):
                cs = 4 * blk_i
                nc.gpsimd.tensor_copy(out=stats8[:, cs:cs + 4], in_=st_tiles[(0, blk_i)][:])
                for p in range(1, PAIRS):
                    nc.gpsimd.tensor_add(out=stats8[:, cs:cs + 4], in0=stats8[:, cs:cs + 4],
                                         in1=st_tiles[(p, blk_i)][:])
            pf2 = misc_ps.tile([128, 512], f32, tag="misc")
            if do_post:
                nc.tensor.matmul(pf2[0:64, 0:8], ii[:], stats8[:], start=True, stop=True)
            st8f = const.tile([64, 8], f32, tag="st8f")
            ar2_sb = const.tile([64, 8], f32, tag="ar2sb")
            if do_post:
                nc.vector.tensor_copy(out=st8f[:], in_=pf2[0:64, 0:8])
                ar2_in = dram.tile([64, 8], f32, tag="ar2i")
                ar2_out = dram.tile([64, 8], f32, tag="ar2o")
                nc.sync.dma_start(out=ar2_in[:], in_=st8f[:])
                nc.gpsimd.collective_compute(
                    "AllReduce", Alu.add, replica_groups=[list(range(N_CORES))],
                    ins=[ar2_in[:].opt()], outs=[ar2_out[:].opt()])
                nc.sync.dma_start(out=ar2_sb[:], in_=ar2_out[:])
                if debug:
                    nc.sync.dma_start(out=dbg['stats8'][:], in_=ar2_sb[:])

            # per-block scale/shift
            scs = []
            bnw = const.tile([64, 16], f32, tag="bnw")
            for blk_i, BL in enumerate((BLK_A, BLK_B) if do_post else ()):
                c0 = 4 * blk_i
                w0 = 8 * blk_i
                rN = 1.0 / BL['Ndp']
                nc.vector.tensor_add(out=bnw[:, w0:w0 + 1], in0=ar2_sb[:, c0:c0 + 1], in1=ar2_sb[:, c0 + 2:c0 + 3])
                nc.vector.tensor_scalar_mul(out=bnw[:, w0:w0 + 1], in0=bnw[:, w0:w0 + 1], scalar1=rN)  # mu
                nc.vector.tensor_add(out=bnw[:, w0 + 1:w0 + 2], in0=ar2_sb[:, c0 + 1:c0 + 2], in1=ar2_sb[:, c0 + 3:c0 + 4])
                nc.vector.tensor_scalar_mul(out=bnw[:, w0 + 1:w0 + 2], in0=bnw[:, w0 + 1:w0 + 2], scalar1=rN)  # E[g^2]
                nc.vector.tensor_mul(out=bnw[:, w0 + 2:w0 + 3], in0=bnw[:, w0:w0 + 1], in1=bnw[:, w0:w0 + 1])
                nc.vector.tensor_sub(out=bnw[:, w0 + 3:w0 + 4], in0=bnw[:, w0 + 1:w0 + 2], in1=bnw[:, w0 + 2:w0 + 3])  # var
                nc.scalar.activation(out=bnw[:, w0 + 4:w0 + 5], in_=bnw[:, w0 + 3:w0 + 4], func=Act.Sqrt,
                                     bias=eps_sb[:])
                nc.vector.reciprocal(out=bnw[:, w0 + 5:w0 + 6], in_=bnw[:, w0 + 4:w0 + 5])
                nc.vector.tensor_mul(out=bnw[:, w0 + 6:w0 + 7], in0=gamma_sb[:], in1=bnw[:, w0 + 5:w0 + 6])  # scale
                nc.vector.tensor_mul(out=bnw[:, w0 + 7:w0 + 8], in0=bnw[:, w0:w0 + 1], in1=bnw[:, w0 + 6:w0 + 7])
                sc128 = const.tile([128, 1], f32, tag=f"sc128_{blk_i}")
                sh128 = const.tile([128, 1], f32, tag=f"sh128_{blk_i}")
                nc.vector.tensor_copy(out=sc128[0:64, :], in_=bnw[:, w0 + 6:w0 + 7])
                nc.vector.tensor_copy(out=sc128[64:128, :], in_=bnw[:, w0 + 6:w0 + 7])
                nc.vector.tensor_sub(out=sh128[0:64, :], in0=beta_sb[:], in1=bnw[:, w0 + 7:w0 + 8])
                nc.vector.tensor_copy(out=sh128[64:128, :], in_=sh128[0:64, :])
                scs.append((sc128, sh128))

            # ---------------- bn apply -> u tiles ----------------
            u_tiles = {}
            for p in range(PAIRS if do_post else 0):
                for blk_i, BL in enumerate((BLK_A, BLK_B)):
                    EW, AW, PL = BL['EW'], BL['AW'], BL['PL']
                    tagb = f"b{blk_i}"
                    sc128, sh128 = scs[blk_i]
                    gt = g_tiles[(p, blk_i)]
                    bnv = tmp_p.tile([128, BLK_A["AW"]], f16, tag="t1")
                    bmin = tmp_p.tile([128, BLK_A["AW"]], f16, tag="t2")
                    bmax = tmp_p.tile([128, BLK_A["AW"]], f16, tag="t3")
                    nc.vector.tensor_scalar(out=bnv[:, 0:PL], in0=gt[:, 0:PL],
                                            scalar1=sc128[:], scalar2=sh128[:],
                                            op0=Alu.mult, op1=Alu.add)
                    nc.vector.tensor_scalar_min(out=bmin[:, 0:PL], in0=bnv[:, 0:PL], scalar1=0.0)
                    nc.vector.tensor_scalar_max(out=bmax[:, 0:PL], in0=bnv[:, 0:PL], scalar1=0.0)
                    nc.scalar.activation(out=bmin[:, 0:PL], in_=bmin[:, 0:PL], func=Act.Exp)
                    ut = gu_p.tile([128, AW], f16, tag="u" + tagb)
                    nc.vector.scalar_tensor_tensor(out=ut[:, 0:PL], in0=bmax[:, 0:PL],
                                                   scalar=-1.0, in1=bmin[:, 0:PL],
                                                   op0=Alu.add, op1=Alu.add)
                    u_tiles[(p, blk_i)] = ut
                    if debug and p == 0 and blk_i == 0:
                        nc.sync.dma_start(out=dbg['g_A'][:], in_=gt[:])
                        nc.sync.dma_start(out=dbg['u_A'][:], in_=ut[:])

            # ---------------- final assembly ----------------
            for p in range(PAIRS if do_post else 0):
                for ci in range(6144 // CH):
                    cs_u, ce_u = ci * CH, (ci + 1) * CH
                    if (p, ci) in stage_tiles:
                        stage = stage_tiles[(p, ci)]
                    else:
                        stage = stage_p.tile([128, CH], f32, tag="stage")
                        _emit_ops(p, ci, stage, 'copy')
                    _emit_ops(p, ci, stage, 'add')
                    nc.sync.dma_start(out=out_t[2 * p, :, cs_u:ce_u], in_=stage[0:64, :])
                    nc.sync.dma_start(out=out_t[2 * p + 1, :, cs_u:ce_u], in_=stage[64:128, :])

    nc.finalize()
    return nc


# ---------------------------------------------------------------------------
# public entry
# ---------------------------------------------------------------------------

_cache = {}
_lock = threading.Lock()


def _get_program(debug=False):
    with _lock:
        key = bool(debug)
        if key not in _cache:
            _cache[key] = _build_program(debug=debug)
        return _cache[key]


def kernel(x, conv_v, conv_g, conv_b, bn_gamma, bn_beta, _debug=False, _trace=False):
    x = np.ascontiguousarray(np.asarray(x, dtype=np.float32))
    conv_v = np.asarray(conv_v, dtype=np.float32)
    conv_g = np.asarray(conv_g, dtype=np.float32)
    conv_b = np.asarray(conv_b, dtype=np.float32)
    bn_gamma = np.asarray(bn_gamma, dtype=np.float32)
    bn_beta = np.asarray(bn_beta, dtype=np.float32)

    nc = _get_program(debug=_debug)
    in_maps = []
    for ci in range(N_CORES):
        in_maps.append(dict(
            x=x[ci * BPC:(ci + 1) * BPC],
            conv_v=conv_v, conv_g=conv_g, conv_b=conv_b,
            bn_gamma=bn_gamma, bn_beta=bn_beta,
        ))
    res = run_bass_kernel_spmd(nc, in_maps, core_ids=list(range(N_CORES)),
                               trace=_trace)
    out = np.concatenate([res.results[ci]["out"] for ci in range(N_CORES)], axis=0)
    if _debug or _trace:
        return out, res
    return out

